# revision 21
# baseline (speedup 1.0000x reference)
"""Trainium2 Bass kernel for nn_Block_87428354277599 (sinkhorn-attention transformer block).

Self-contained: hardcodes shapes/sharding. kernel(**inputs) -> (2, 2048, 384) f32.

Design (8 cores, SPMD, uniform program):
- 12 (batch, head) units on 16 slots: cores 0-3 = batch 0, 4-7 = batch 1;
  in-group rank g slots: g0:(h0,h1) g1:(h2,h3) g2:(h4,-) g3:(h5,-);
  '-' slots run on zero weights, output masked at the receiver's folded
  projection weights.
- Sinkhorn on S = exp(row-softmax(causal scores)) converges after ONE
  u-update (validated ~2e-4): pi = S / rowsum(S). With S' = S-1 strictly
  lower-triangular:  y_i = (w + S'@v)_i / (T + rowsum(S')_i),  w = colsum(v).
- Everything stays in the TRANSPOSED layout (partition = key j, free =
  query i): scores^T via swapped matmul, z = rowsum(e) via PE ones-matvec,
  softmax-normalize via PE-broadcast rz + vector multiply, S' = expm1(p)
  via a 2-op vector polynomial p*(1+p/2) (exact scalar exp for i<128 where
  p can be ~1), numerator AND denominator fused into one PE pass by
  augmenting v with a ones column. No PE transposes of S, no DRAM bounces.
- 1/(T+r') is linearized to (T-r')/T^2 (r' <= ~3, error 2e-6); 1/z uses
  the single-op DVE approx reciprocal (feeds a bf16 cast anyway).
- bf16 throughout; f32 accumulation in PSUM and for LN stats rows.
- Group-local AllToAll (4 cores per batch) ships y^T bf16; tail
  (proj+LN2+MLP) is row-sharded 512 tokens/core with LN folded into the
  matmuls via host-precomputed rank-1 corrections.
"""

import numpy as np

import concourse.bacc as bacc
import concourse.mybir as mybir
from concourse.tile import TileContext
from concourse.bass_utils import run_bass_kernel_spmd

F32 = mybir.dt.float32
BF16 = mybir.dt.bfloat16
AF = mybir.ActivationFunctionType
ALU = mybir.AluOpType

B, T, C, H, HD = 2, 2048, 384, 6, 64
CP1 = C + 1
N_CORES = 8
NT = T // 128   # 16
NC4 = T // 512  # 4
EPS = 1e-5

_COMPILED = {}


def build_program():
    nc = bacc.Bacc(trn_type="TRN2", num_devices=N_CORES)
    mm = nc.tensor.matmul

    def din(name, shape, dt=F32):
        return nc.dram_tensor(name, list(shape), dt, kind="ExternalInput")

    xT_d = din("xT", (3, 128, T), BF16)
    wqk_d = din("wqk", (128, 6 * 128), BF16)
    wv_d = din("wv", (128, 3 * 128), BF16)
    r1qk_d = din("r1qk", (1, 512), BF16)
    r1v_d = din("r1v", (1, 256), BF16)
    onesT_d = din("onesT", (1, T), BF16)
    c1qk_d = din("c1qk", (128, 2))
    c1v_d = din("c1v", (128, 1))
    ident_d = din("ident", (64, 64), BF16)
    onesc_d = din("onesc", (128, 1), BF16)
    ones65_d = din("ones65", (65, 128), BF16)
    tcol_d = din("tcol", (1, 1))
    sbias_d = din("sbias", (1, 2))
    epsc_d = din("epsc", (1, 1))
    wproj_d = din("wproj", (2, 64, 24 * 128), BF16)
    bproj_d = din("bproj", (128, 3))
    wf_d = din("wf", (128, 36 * 128), BF16)
    r2f_d = din("r2f", (1, 1536), BF16)
    c2b_d = din("c2b", (128, 12))
    wf2_d = din("wf2", (128, 36 * 128), BF16)
    bfc2_d = din("bfc2", (128, 3))
    out_d = nc.dram_tensor("oT", [C, 512], F32, kind="ExternalOutput")

    with TileContext(nc) as tc, nc.allow_low_precision(reason="bf16 kernel, validated ~6e-3 < 2e-2 gate"):
        with (
            tc.tile_pool(name="const", bufs=1) as cpool,
            tc.tile_pool(name="dram", bufs=1, space="DRAM") as dpool,
            tc.tile_pool(name="ps_mm", bufs=2, space="PSUM") as ppm,
            tc.tile_pool(name="ps_z", bufs=2, space="PSUM") as ppz,
            tc.tile_pool(name="ps_y", bufs=2, space="PSUM") as ppy,
            tc.tile_pool(name="qk", bufs=1) as qkp,
        ):
            a2a_in = [dpool.tile([8, 64, 512], BF16, name=f"a2a_in{s}") for s in range(2)]
            a2a_out = [dpool.tile([8, 64, 512], BF16, name=f"a2a_out{s}") for s in range(2)]

            ident = cpool.tile([64, 64], BF16, tag="ident", name="ident")
            onesc = cpool.tile([128, 1], BF16, tag="onesc", name="onesc")
            ones65 = cpool.tile([65, 128], BF16, tag="ones65", name="ones65")
            tcol = cpool.tile([1, 1], F32, tag="tcol", name="tcol")
            sbias = cpool.tile([1, 2], F32, tag="sbias", name="sbias")
            epsc = cpool.tile([1, 1], F32, tag="epsc", name="epsc")

            # persistent per-slot activations (base-partition-0, bf16)
            qT = [qkp.tile([64, T], BF16, tag=f"qT{s}", name=f"qT{s}") for s in range(2)]
            kT = [qkp.tile([64, T], BF16, tag=f"kT{s}", name=f"kT{s}") for s in range(2)]
            vAug = [qkp.tile([128, NT * 65], BF16, tag=f"vAug{s}", name=f"vAug{s}") for s in range(2)]

            # ---------------- phase 1: LN1 stats + QKV ----------------
            with (
                tc.tile_pool(name="xt", bufs=1) as xp,
                tc.tile_pool(name="sq", bufs=2) as sqp,
                tc.tile_pool(name="st", bufs=4) as stp,
                tc.tile_pool(name="ps_tr", bufs=2, space="PSUM") as ppt,
            ):
                xT = [xp.tile([128, T], BF16, tag=f"xt{kc}", name=f"xt{kc}") for kc in range(3)]
                for kc in range(3):
                    nc.sync.dma_start(out=xT[kc][:, :], in_=xT_d[kc, :, :])
                nc.sync.dma_start(out=ident[:, :], in_=ident_d[:, :])
                nc.sync.dma_start(out=onesc[:, :], in_=onesc_d[:, :])
                nc.sync.dma_start(out=ones65[:, :], in_=ones65_d[:, :])
                nc.sync.dma_start(out=tcol[:, :], in_=tcol_d[:, :])
                nc.sync.dma_start(out=sbias[:, :], in_=sbias_d[:, :])
                nc.sync.dma_start(out=epsc[:, :], in_=epsc_d[:, :])
                wqkall = xp.tile([128, 6 * 128], BF16, tag="wqkall", name="wqkall")
                wvall = xp.tile([128, 3 * 128], BF16, tag="wvall", name="wvall")
                wqk = [[wqkall[:, (s * 3 + kc) * 128:(s * 3 + kc + 1) * 128] for kc in range(3)] for s in range(2)]
                wv = [wvall[:, kc * 128:(kc + 1) * 128] for kc in range(3)]
                r1qk = xp.tile([1, 512], BF16, tag="r1qk", name="r1qk")
                r1v = xp.tile([1, 256], BF16, tag="r1v", name="r1v")
                onesT = xp.tile([1, T], BF16, tag="onesT", name="onesT")
                nc.sync.dma_start(out=onesT[:, :], in_=onesT_d[:, :])
                c1qk = xp.tile([128, 2], F32, tag="c1qk", name="c1qk")
                c1v = xp.tile([128, 1], F32, tag="c1v", name="c1v")
                nc.sync.dma_start(out=r1qk[:, :], in_=r1qk_d[:, :])
                nc.sync.dma_start(out=r1v[:, :], in_=r1v_d[:, :])
                nc.sync.dma_start(out=c1qk[:, :], in_=c1qk_d[:, :])
                nc.sync.dma_start(out=c1v[:, :], in_=c1v_d[:, :])
                nc.sync.dma_start(out=wqkall[:, :], in_=wqk_d[:, :])
                nc.sync.dma_start(out=wvall[:, :], in_=wv_d[:, :])

                mu16 = xp.tile([1, T], BF16, tag="mu16", name="mu16")
                mu_row = xp.tile([1, T], F32, tag="mu_row", name="mu_row")
                rstd_bc = xp.tile([128, T], F32, tag="rstd_bc", name="rstd_bc")

                # ---- per-chunk stats chains pipelined & interleaved with QKV ----
                qk_c = [xp.tile([128, T], BF16, tag=f"qk_c{s}", name=f"qk_c{s}") for s in range(2)]
                v_c = xp.tile([128, T], BF16, tag="v_c", name="v_c")
                vA = [xp.tile([64, T], BF16, tag=f"vA{s}", name=f"vA{s}") for s in range(2)]
                for s in range(2):
                    nc.gpsimd.memset(vAug[s][:, :], 1.0)

                def stats_chunk(c4):
                    sl = slice(c4 * 512, (c4 + 1) * 512)
                    ps = ppz.tile([1, 512], F32, tag="z", name="z")
                    for kc in range(3):
                        mm(ps[0:1, :], onesc[:, :], xT[kc][:, sl], start=(kc == 0), stop=(kc == 2))
                    nc.scalar.activation(mu_row[0:1, sl], ps[0:1, :], AF.Identity,
                                         bias=sbias[0:1, 0:1], scale=1.0 / CP1)
                    ps2 = ppz.tile([1, 512], F32, tag="z", name="z")
                    for kc in range(3):
                        sq = sqp.tile([128, 512], BF16, tag="sq", name="sq")
                        nc.scalar.square(sq[:, :], xT[kc][:, sl])
                        mm(ps2[0:1, :], onesc[:, :], sq[:, :], start=(kc == 0), stop=(kc == 2))
                    msq = stp.tile([1, 512], F32, tag="msq", name="msq")
                    nc.scalar.activation(msq[0:1, :], ps2[0:1, :], AF.Identity,
                                         bias=sbias[0:1, 1:2], scale=1.0 / CP1)
                    var = stp.tile([1, 512], F32, tag="var", name="var")
                    nc.vector.tensor_tensor(var[0:1, :], mu_row[0:1, sl], mu_row[0:1, sl], ALU.mult)
                    nc.vector.tensor_tensor(var[0:1, :], msq[0:1, :], var[0:1, :], ALU.subtract)
                    nc.scalar.activation(var[0:1, :], var[0:1, :], AF.Sqrt, bias=epsc[0:1, 0:1])
                    rstd = stp.tile([1, 512], F32, tag="rstd", name="rstd")
                    rscr = stp.tile([1, 512], F32, tag="rscr", name="rscr")
                    nc.vector.reciprocal_approx_accurate(rstd[0:1, :], var[0:1, :], rscr[0:1, :])
                    rstd16 = stp.tile([1, 512], BF16, tag="rstd16", name="rstd16")
                    nc.vector.tensor_copy(rstd16[0:1, :], rstd[0:1, :])
                    nc.vector.tensor_copy(mu16[0:1, sl], mu_row[0:1, sl])
                    bps = ppm.tile([128, 512], F32, tag="mm", name="mm")
                    mm(bps[:, :], ones65[0:1, :], rstd16[0:1, :], start=True, stop=True)
                    nc.vector.tensor_copy(rstd_bc[:, sl], bps[:, :])

                def qkv_mat(c4, dst, lhsT_chunks, r1a, r1b, c1col):
                    sl = slice(c4 * 512, (c4 + 1) * 512)
                    ps = ppm.tile([128, 512], F32, tag="mm", name="mm")
                    for kc in range(3):
                        mm(ps[:, :], lhsT_chunks[kc], xT[kc][:, sl],
                           start=(kc == 0), stop=False)
                    mm(ps[:, :], r1a, mu16[0:1, sl], start=False, stop=False)
                    mm(ps[:, :], r1b, onesT[0:1, sl], start=False, stop=True)
                    nc.vector.tensor_tensor(dst[:, sl], ps[:, :], rstd_bc[:, sl], ALU.mult)
                    nc.vector.tensor_scalar(dst[:, sl], dst[:, sl], c1col, None, ALU.add)

                def qkv_chunk(c4):
                    sl = slice(c4 * 512, (c4 + 1) * 512)
                    for s in range(2):
                        qkv_mat(c4, qk_c[s], wqk[s],
                                r1qk[0:1, s * 128:(s + 1) * 128],
                                r1qk[0:1, 256 + s * 128:256 + (s + 1) * 128],
                                c1qk[:, s:s + 1])
                    qkv_mat(c4, v_c, wv, r1v[0:1, 0:128], r1v[0:1, 128:256], c1v[:, 0:1])
                    for s in range(2):
                        nc.sync.dma_start(out=qT[s][:, sl], in_=qk_c[s][0:64, sl])
                        nc.sync.dma_start(out=kT[s][:, sl], in_=qk_c[s][64:128, sl])
                        nc.sync.dma_start(out=vA[s][:, sl], in_=v_c[s * 64:(s + 1) * 64, sl])
                    for s in range(2):
                        for jt in range(4 * c4, 4 * c4 + 4):
                            tr = ppt.tile([128, 64], BF16, tag="tr", name="tr")
                            nc.tensor.transpose(tr[:, :], vA[s][:, jt * 128:(jt + 1) * 128], ident[:, :])
                            nc.vector.tensor_copy(vAug[s][:, jt * 65:jt * 65 + 64], tr[:, :])

                stats_chunk(0)
                stats_chunk(1)
                qkv_chunk(0)
                stats_chunk(2)
                qkv_chunk(1)
                stats_chunk(3)
                qkv_chunk(2)
                qkv_chunk(3)

            # ---------------- phase 2: attention (transposed layout) ----------------
            with (
                tc.tile_pool(name="sp", bufs=1) as spp,
                tc.tile_pool(name="att", bufs=2) as amp,
                tc.tile_pool(name="attp", bufs=1) as am1,
                tc.tile_pool(name="ps_b", bufs=2, space="PSUM") as ppb,
            ):
                # spt tiles: per (slot, jt), columns = global i in [jt*128, T)
                spt = [[spp.tile([128, T - jt * 128], BF16, tag=f"spt{s}_{jt}", name=f"spt{s}_{jt}")
                        for jt in range(NT)] for s in range(2)]
                w65 = [am1.tile([65, 1], F32, tag=f"w65_{s}", name=f"w65_{s}") for s in range(2)]
                prewarm = am1.tile([1, 1], F32, tag="prewarm", name="prewarm")

                # w = colsum(vAug) over all j  (per slot, [65,1])
                for s in range(2):
                    wps = ppy.tile([65, 512], F32, tag="y", name="y")
                    for jt in range(NT):
                        mm(wps[:, 0:1], vAug[s][:, jt * 65:(jt + 1) * 65], onesc[:, :],
                           start=(jt == 0), stop=(jt == NT - 1))
                    nc.vector.tensor_copy(w65[s][:, :], wps[:, 0:1])

                for s in range(2):
                    for c4 in range(NC4):
                        ic0, ic1 = c4 * 512, (c4 + 1) * 512
                        njt = 4 * c4 + 4
                        # stage A: scores^T + exp + diag mask
                        for jt in range(njt):
                            j0 = jt * 128
                            lo = max(ic0, j0)
                            w_ = ic1 - lo
                            ps = ppm.tile([128, 512], F32, tag="mm", name="mm")
                            mm(ps[:, 0:w_], kT[s][:, j0:j0 + 128], qT[s][:, lo:ic1],
                               start=True, stop=True)
                            dst = spt[s][jt][:, lo - j0:ic1 - j0]
                            nc.scalar.activation(dst, ps[:, 0:w_], AF.Exp, scale=0.125)
                            if j0 + 128 > lo:  # tile straddles the diagonal: zero j > i
                                nc.gpsimd.affine_select(
                                    out=dst, in_=dst, compare_op=ALU.is_ge, fill=0.0,
                                    base=lo - j0, channel_multiplier=-1, pattern=[[1, w_]])
                        if c4 == 0:
                            # full path: z, normalize, expm1 (exact / poly), yAug on S'
                            zps = ppz.tile([1, 512], F32, tag="z", name="z")
                            for jt in range(njt):
                                j0 = jt * 128
                                lo = max(ic0, j0)
                                mm(zps[0:1, lo - ic0:512], onesc[:, :],
                                   spt[s][jt][:, lo - j0:ic1 - j0],
                                   start=(jt == 0), stop=(jt == njt - 1))
                            rz = amp.tile([1, 512], F32, tag="rz", name="rz")
                            nc.vector.reciprocal_approx_fast(out=rz[0:1, :], in_=zps[0:1, :])
                            rz16 = amp.tile([1, 512], BF16, tag="rz16", name="rz16")
                            nc.vector.tensor_copy(rz16[0:1, :], rz[0:1, :])
                            bps = ppb.tile([128, 512], F32, tag="bc", name="bc")
                            mm(bps[:, :], ones65[0:1, :], rz16[0:1, :], start=True, stop=True)
                            rzbc = amp.tile([128, 512], BF16, tag="rzbc", name="rzbc")
                            nc.scalar.copy(rzbc[:, :], bps[:, :])
                            for jt in range(njt):
                                j0 = jt * 128
                                lo = max(ic0, j0)
                                w_ = ic1 - lo
                                piece = spt[s][jt][:, lo - j0:ic1 - j0]
                                nc.vector.tensor_tensor(piece, piece, rzbc[:, lo - ic0:512], ALU.mult)
                                if jt == 0:
                                    ex = spt[s][0][:, 0:128]
                                    nc.scalar.activation(ex, ex, AF.Exp)
                                    nc.vector.tensor_scalar(ex, ex, -1.0, None, ALU.add)
                                    po = spt[s][0][:, 128:512]
                                    t1 = amp.tile([128, 512], BF16, tag="t1", name="t1")
                                    nc.vector.tensor_scalar(t1[:, 0:384], po, 0.5, 1.0, ALU.mult, ALU.add)
                                    nc.vector.tensor_tensor(po, po, t1[:, 0:384], ALU.mult)
                                else:
                                    t1 = amp.tile([128, 512], BF16, tag="t1", name="t1")
                                    nc.vector.tensor_scalar(t1[:, 0:w_], piece, 0.5, 1.0, ALU.mult, ALU.add)
                                    nc.vector.tensor_tensor(piece, piece, t1[:, 0:w_], ALU.mult)
                            yps = ppy.tile([65, 512], F32, tag="y", name="y")
                            for jt in range(njt):
                                j0 = jt * 128
                                lo = max(ic0, j0)
                                mm(yps[:, lo - ic0:512], vAug[s][:, jt * 65:(jt + 1) * 65],
                                   spt[s][jt][:, lo - j0:ic1 - j0],
                                   start=(jt == 0), stop=(jt == njt - 1))
                            a16 = amp.tile([65, 512], BF16, tag="a16", name="a16")
                            nc.vector.tensor_scalar(a16[64:65, :], yps[64:65, :],
                                                    float(T), -1.0 / (T * T), ALU.subtract, ALU.mult)
                            bps2 = ppb.tile([128, 512], F32, tag="bc", name="bc")
                            mm(bps2[:, :], ones65[64:65, :], a16[64:65, :], start=True, stop=True)
                            abc = amp.tile([64, 512], BF16, tag="abc", name="abc")
                            nc.scalar.copy(abc[:, :], bps2[0:64, :])
                            y16 = amp.tile([64, 512], BF16, tag="y16", name="y16")
                            nc.vector.tensor_scalar(y16[:, :], yps[0:64, :], w65[s][0:64, 0:1], None, ALU.add)
                            nc.vector.tensor_tensor(y16[:, :], y16[:, :], abc[:, :], ALU.mult)
                        else:
                            # identity region (i>=512): S' = p = e*rz, so rz factors out:
                            # yAug on e directly; row 64 = z; denom = T+1 exactly
                            yps = ppy.tile([65, 512], F32, tag="y", name="y")
                            for jt in range(njt):
                                j0 = jt * 128
                                lo = max(ic0, j0)
                                mm(yps[:, lo - ic0:512], vAug[s][:, jt * 65:(jt + 1) * 65],
                                   spt[s][jt][:, lo - j0:ic1 - j0],
                                   start=(jt == 0), stop=(jt == njt - 1))
                            z16 = amp.tile([65, 512], BF16, tag="z16_65", name="z16_65")
                            nc.vector.tensor_copy(z16[64:65, :], yps[64:65, :])
                            bps = ppb.tile([128, 512], F32, tag="bc", name="bc")
                            mm(bps[:, :], ones65[64:65, :], z16[64:65, :], start=True, stop=True)
                            rzbcf = amp.tile([128, 512], F32, tag="rzbcf", name="rzbcf")
                            nc.vector.reciprocal_approx_fast(out=rzbcf[:, :], in_=bps[:, :])
                            y16 = amp.tile([64, 512], BF16, tag="y16", name="y16")
                            nc.vector.tensor_tensor(y16[:, :], yps[0:64, :], rzbcf[0:64, :], ALU.mult)
                            nc.vector.tensor_scalar(y16[:, :], y16[:, :], w65[s][0:64, 0:1],
                                                    1.0 / (T + 1), ALU.add, ALU.mult)
                        for grp in range(2):
                            nc.sync.dma_start(out=a2a_in[s][grp * 4 + c4, :, :], in_=y16[:, :])
                    if s == 1:  # prewarm the Sqrt act-table set during the a2a wait
                        nc.scalar.activation(prewarm[0:1, :], epsc[0:1, :], AF.Sqrt)
                    # per-slot AllToAll: slot 0's transfer overlaps slot 1's compute
                    nc.gpsimd.collective_compute(
                        "AllToAll", ALU.bypass,
                        replica_groups=[list(range(N_CORES))],
                        ins=[a2a_in[s].opt()],
                        outs=[a2a_out[s].opt()],
                    )

            # ---------------- phase 4: proj + LN2 + MLP (512 tokens/core) ----------------
            with (
                tc.tile_pool(name="tail", bufs=1) as tp,
                tc.tile_pool(name="ps_b2", bufs=2, space="PSUM") as ppb2,
            ):
                wpallA = tp.tile([64, 24 * 128], BF16, tag="wpallA", name="wpallA")
                wpallB = tp.tile([64, 24 * 128], BF16, tag="wpallB", name="wpallB")
                wprojA = [[wpallA[:, (sl_ * 3 + ec) * 128:(sl_ * 3 + ec + 1) * 128] for ec in range(3)] for sl_ in range(8)]
                wprojB = [[wpallB[:, (sl_ * 3 + ec) * 128:(sl_ * 3 + ec + 1) * 128] for ec in range(3)] for sl_ in range(8)]
                bproj = tp.tile([128, 3], F32, tag="bproj", name="bproj")
                wfall = tp.tile([128, 36 * 128], BF16, tag="wfall", name="wfall")
                wf = [[wfall[:, (jc * 3 + kc) * 128:(jc * 3 + kc + 1) * 128] for kc in range(3)] for jc in range(12)]
                r2f = tp.tile([1, 1536], BF16, tag="r2f", name="r2f")
                c2b = tp.tile([128, 12], F32, tag="c2b", name="c2b")
                wf2all = tp.tile([128, 36 * 128], BF16, tag="wf2all", name="wf2all")
                wf2 = [[wf2all[:, (ec * 12 + kc) * 128:(ec * 12 + kc + 1) * 128] for kc in range(12)] for ec in range(3)]
                bfc2 = tp.tile([128, 3], F32, tag="bfc2", name="bfc2")
                nc.sync.dma_start(out=wpallA[:, :], in_=wproj_d[0, :, :])
                nc.sync.dma_start(out=wpallB[:, :], in_=wproj_d[1, :, :])
                nc.sync.dma_start(out=bproj[:, :], in_=bproj_d[:, :])
                nc.sync.dma_start(out=wfall[:, :], in_=wf_d[:, :])
                nc.sync.dma_start(out=r2f[:, :], in_=r2f_d[:, :])
                nc.sync.dma_start(out=c2b[:, :], in_=c2b_d[:, :])
                nc.sync.dma_start(out=wf2all[:, :], in_=wf2_d[:, :])
                nc.sync.dma_start(out=bfc2[:, :], in_=bfc2_d[:, :])

                stkA = [tp.tile([64, 512], BF16, tag=f"stkA{sl_}", name=f"stkA{sl_}") for sl_ in range(8)]
                stkB = [tp.tile([64, 512], BF16, tag=f"stkB{sl_}", name=f"stkB{sl_}") for sl_ in range(8)]
                for sl_ in range(8):
                    nc.sync.dma_start(out=stkA[sl_][:, :], in_=a2a_out[0][sl_, :, :])
                for sl_ in range(8):
                    nc.sync.dma_start(out=stkB[sl_][:, :], in_=a2a_out[1][sl_, :, :])

                hT = [tp.tile([128, 512], BF16, tag=f"ht{ec}", name=f"ht{ec}") for ec in range(3)]
                mu2ps = ppz.tile([1, 512], F32, tag="z", name="z")
                msq2ps = ppz.tile([1, 512], F32, tag="z", name="z")
                for ec in range(3):
                    ps = ppm.tile([128, 512], F32, tag="mm", name="mm")
                    for sl_ in range(8):
                        mm(ps[:, :], wprojA[sl_][ec], stkA[sl_][:, :],
                           start=(sl_ == 0), stop=False)
                    for sl_ in range(8):
                        mm(ps[:, :], wprojB[sl_][ec], stkB[sl_][:, :],
                           start=False, stop=(sl_ == 7))
                    nc.scalar.activation(hT[ec][:, :], ps[:, :], AF.Identity,
                                         bias=bproj[:, ec:ec + 1], scale=1.0)
                    mm(mu2ps[0:1, :], onesc[:, :], hT[ec][:, :], start=(ec == 0), stop=(ec == 2))
                    scr2 = tp.tile([128, 512], BF16, tag="scr2", name="scr2")
                    nc.scalar.square(scr2[:, :], hT[ec][:, :])
                    mm(msq2ps[0:1, :], onesc[:, :], scr2[:, :], start=(ec == 0), stop=(ec == 2))
                mu2r = tp.tile([1, 512], F32, tag="mu2r", name="mu2r")
                nc.scalar.activation(mu2r[0:1, :], mu2ps[0:1, :], AF.Identity,
                                     bias=sbias[0:1, 0:1], scale=1.0 / CP1)
                msq2r = tp.tile([1, 512], F32, tag="msq2r", name="msq2r")
                nc.scalar.activation(msq2r[0:1, :], msq2ps[0:1, :], AF.Identity,
                                     bias=sbias[0:1, 1:2], scale=1.0 / CP1)
                v2r = tp.tile([1, 512], F32, tag="v2r", name="v2r")
                nc.vector.tensor_tensor(v2r[0:1, :], mu2r[0:1, :], mu2r[0:1, :], ALU.mult)
                nc.vector.tensor_tensor(v2r[0:1, :], msq2r[0:1, :], v2r[0:1, :], ALU.subtract)
                nc.scalar.activation(v2r[0:1, :], v2r[0:1, :], AF.Sqrt, bias=epsc[0:1, 0:1])
                rstd2r = tp.tile([1, 512], F32, tag="rstd2r", name="rstd2r")
                r2scr = tp.tile([1, 512], F32, tag="r2scr", name="r2scr")
                nc.vector.reciprocal_approx_accurate(rstd2r[0:1, :], v2r[0:1, :], r2scr[0:1, :])
                rstd216 = tp.tile([1, 512], BF16, tag="rstd216", name="rstd216")
                nc.vector.tensor_copy(rstd216[0:1, :], rstd2r[0:1, :])
                mu216 = tp.tile([1, 512], BF16, tag="mu216", name="mu216")
                nc.vector.tensor_copy(mu216[0:1, :], mu2r[0:1, :])
                psm = ppb2.tile([128, 512], F32, tag="bc2", name="bc2")
                mm(psm[:, :], ones65[0:1, :], mu216[0:1, :], start=True, stop=True)
                mu2bc = tp.tile([128, 512], F32, tag="mu2bc", name="mu2bc")
                nc.scalar.copy(mu2bc[:, :], psm[:, :])
                ps = ppb2.tile([128, 512], F32, tag="bc2", name="bc2")
                mm(ps[:, :], ones65[0:1, :], rstd216[0:1, :], start=True, stop=True)
                rstd2bc = tp.tile([128, 512], F32, tag="rstd2bc", name="rstd2bc")
                nc.vector.tensor_copy(rstd2bc[:, :], ps[:, :])
                hs = [tp.tile([128, 512], BF16, tag=f"hs{ec}", name=f"hs{ec}") for ec in range(3)]
                for ec in range(3):
                    hd = tp.tile([128, 512], BF16, tag="hd", name="hd")
                    nc.vector.tensor_tensor(hd[:, :], hT[ec][:, :], mu2bc[:, :], ALU.subtract)
                    nc.vector.tensor_tensor(hs[ec][:, :], hd[:, :], rstd2bc[:, :], ALU.mult)

                mT = [tp.tile([128, 512], BF16, tag=f"mt{jc}", name=f"mt{jc}") for jc in range(12)]
                for jc in range(12):
                    ps = ppm.tile([128, 512], F32, tag="mm", name="mm")
                    for kc in range(3):
                        mm(ps[:, :], wf[jc][kc], hs[kc][:, :], start=(kc == 0), stop=False)
                    mm(ps[:, :], r2f[0:1, jc * 128:(jc + 1) * 128], rstd216[0:1, :], start=False, stop=True)
                    nc.scalar.activation(mT[jc][:, :], ps[:, :], AF.Gelu,
                                         bias=c2b[:, jc:jc + 1], scale=1.0)
                for ec in range(3):
                    ps = ppm.tile([128, 512], F32, tag="mm", name="mm")
                    for kc in range(12):
                        mm(ps[:, :], wf2[ec][kc], mT[kc][:, :],
                           start=(kc == 0), stop=(kc == 11))
                    oT = tp.tile([128, 512], F32, tag=f"ot{ec}", name=f"ot{ec}")
                    nc.scalar.activation(oT[:, :], ps[:, :], AF.Identity,
                                         bias=bfc2[:, ec:ec + 1], scale=1.0)
                    nc.sync.dma_start(out=out_d[ec * 128:(ec + 1) * 128, :], in_=oT[:, :])

    nc.compile()
    return nc


def host_prep(inputs):
    import ml_dtypes
    bf16 = ml_dtypes.bfloat16

    x = np.asarray(inputs["x"], np.float32)
    t = float(np.asarray(inputs["t"]).reshape(-1)[0])
    w1 = np.asarray(inputs["ln1_w"], np.float32); b1 = np.asarray(inputs["ln1_b"], np.float32)
    Wa = np.asarray(inputs["attn_w"], np.float32); ba = np.asarray(inputs["attn_b"], np.float32)
    Wp_ = w1[:, None] * Wa
    c1 = b1 @ Wa + ba
    Wa_main, Wa_trow = Wp_[:C], Wp_[C]
    s1 = Wp_[:C].sum(axis=0)
    w2 = np.asarray(inputs["ln2_w"], np.float32); b2 = np.asarray(inputs["ln2_b"], np.float32)
    Wf = np.asarray(inputs["fc_w"], np.float32); bf_ = np.asarray(inputs["fc_b"], np.float32)
    Wf_p = w2[:, None] * Wf
    c2 = b2 @ Wf + bf_
    Wf_main, Wf_trow = Wf_p[:C], Wf_p[C]
    s2f = Wf_p[:C].sum(axis=0)
    Wpj = np.asarray(inputs["proj_w"], np.float32); bpj = np.asarray(inputs["proj_b"], np.float32)
    Wf2 = np.asarray(inputs["fc2_w"], np.float32); bf2 = np.asarray(inputs["fc2_b"], np.float32)

    common = {
        "ident": np.eye(64, dtype=bf16),
        "onesc": np.ones((128, 1), bf16),
        "ones65": np.ones((65, 128), bf16),
        "tcol": np.full((1, 1), t, np.float32),
        "sbias": np.array([[t / CP1, t * t / CP1]], np.float32),
        "epsc": np.full((1, 1), EPS, np.float32),
        "bproj": bpj.reshape(3, 128).T.astype(np.float32).copy(),
        "c2b": c2.reshape(12, 128).T.astype(np.float32).copy(),
        "bfc2": bf2.reshape(3, 128).T.astype(np.float32).copy(),
        "r2f": np.ascontiguousarray((t * Wf_trow)[None, :]).astype(bf16),
        "onesT": np.ones((1, T), bf16),
        "wf": np.concatenate([Wf_main[kc * 128:(kc + 1) * 128, jc * 128:(jc + 1) * 128]
                              for jc in range(12) for kc in range(3)], axis=1).astype(bf16),
        "wf2": np.concatenate([Wf2[kc * 128:(kc + 1) * 128, ec * 128:(ec + 1) * 128]
                               for ec in range(3) for kc in range(12)], axis=1).astype(bf16),
    }

    # in-group rank g -> (slot0 head, slot1 head); None = dummy slot
    SLOT_HEADS = {0: (0, 1), 1: (2, 3), 2: (4, None), 3: (5, None)}
    # head -> (sender in-group rank, sender slot)
    HEAD_SRC = {0: (0, 0), 1: (0, 1), 2: (1, 0), 3: (1, 1), 4: (2, 0), 5: (3, 0)}

    in_maps = []
    for c in range(N_CORES):
        b, g = c // 4, c % 4
        m = dict(common)
        m["xT"] = np.ascontiguousarray(x[b].T).astype(bf16).reshape(3, 128, T)
        wproj = np.zeros((2, 64, 24 * 128), np.float32)
        for h in range(H):
            sr, sslot = HEAD_SRC[h]
            for ec in range(3):
                blk = Wpj[h * HD:(h + 1) * HD, ec * 128:(ec + 1) * 128]
                blkc = ((4 * b + sr) * 3 + ec) * 128
                wproj[sslot, :, blkc:blkc + 128] = blk
        m["wproj"] = wproj.astype(bf16)
        wqk = np.zeros((2, 3, 128, 128), np.float32)
        r1qk = np.zeros((1, 512), np.float32)
        c1qk = np.zeros((128, 2), np.float32)
        wv = np.zeros((3, 128, 128), np.float32)
        r1v = np.zeros((1, 256), np.float32)
        c1v = np.zeros((128, 1), np.float32)
        for s in range(2):
            h = SLOT_HEADS[g][s]
            if h is None:
                continue
            cq = slice(h * HD, (h + 1) * HD)
            ck = slice(C + h * HD, C + (h + 1) * HD)
            cv = slice(2 * C + h * HD, 2 * C + (h + 1) * HD)
            for kc in range(3):
                wqk[s, kc, :, 0:64] = Wa_main[kc * 128:(kc + 1) * 128, cq]
                wqk[s, kc, :, 64:128] = Wa_main[kc * 128:(kc + 1) * 128, ck]
                wv[kc, :, s * 64:(s + 1) * 64] = Wa_main[kc * 128:(kc + 1) * 128, cv]
            base = s * 128
            r1qk[0, base:base + 64] = -(Wa_trow + s1)[cq]
            r1qk[0, base + 64:base + 128] = -(Wa_trow + s1)[ck]
            r1qk[0, 256 + base:256 + base + 64] = t * Wa_trow[cq]
            r1qk[0, 256 + base + 64:256 + base + 128] = t * Wa_trow[ck]
            r1v[0, s * 64:(s + 1) * 64] = -(Wa_trow + s1)[cv]
            r1v[0, 128 + s * 64:128 + (s + 1) * 64] = t * Wa_trow[cv]
            c1qk[0:64, s] = c1[cq]; c1qk[64:128, s] = c1[ck]
            c1v[s * 64:(s + 1) * 64, 0] = c1[cv]
        m["wqk"] = np.concatenate([wqk[s, kc] for s in range(2) for kc in range(3)],
                                  axis=1).astype(bf16)
        m["r1qk"] = r1qk.astype(bf16); m["c1qk"] = c1qk
        m["wv"] = np.concatenate([wv[kc] for kc in range(3)], axis=1).astype(bf16)
        m["r1v"] = r1v.astype(bf16); m["c1v"] = c1v
        in_maps.append(m)
    return in_maps


def kernel(**inputs):
    if "nc" not in _COMPILED:
        _COMPILED["nc"] = build_program()
    nc = _COMPILED["nc"]
    in_maps = host_prep(inputs)
    res = run_bass_kernel_spmd(nc, in_maps, list(range(N_CORES)))
    out = np.zeros((B, T, C), np.float32)
    for c in range(N_CORES):
        oT = res.results[c]["oT"]
        b, t0 = c // 4, (c % 4) * 512
        out[b, t0:t0 + 512, :] = oT.T
    return out


# revision 22
# speedup vs baseline: 1.1968x; 1.1968x over previous
"""Trainium2 Bass kernel for nn_Block_87428354277599 (sinkhorn-attention transformer block).

Self-contained: hardcodes shapes/sharding. kernel(**inputs) -> (2, 2048, 384) f32.

Design (8 cores, SPMD, uniform program):
- 12 (batch, head) units on 16 slots: cores 0-3 = batch 0, 4-7 = batch 1;
  in-group rank g slots: g0:(h0,h1) g1:(h2,h3) g2:(h4,-) g3:(h5,-);
  '-' slots run on zero weights, output masked at the receiver's folded
  projection weights.
- Sinkhorn on S = exp(row-softmax(causal scores)) converges after ONE
  u-update (validated ~2e-4): pi = S / rowsum(S). With S' = S-1 strictly
  lower-triangular:  y_i = (w + S'@v)_i / (T + rowsum(S')_i),  w = colsum(v).
- Everything stays in the TRANSPOSED layout (partition = key j, free =
  query i): scores^T via swapped matmul, z = rowsum(e) via PE ones-matvec,
  softmax-normalize via PE-broadcast rz + vector multiply, S' = expm1(p)
  via a 2-op vector polynomial p*(1+p/2) (exact scalar exp for i<128 where
  p can be ~1), numerator AND denominator fused into one PE pass by
  augmenting v with a ones column. No PE transposes of S, no DRAM bounces.
- 1/(T+r') is linearized to (T-r')/T^2 (r' <= ~3, error 2e-6); 1/z uses
  the single-op DVE approx reciprocal (feeds a bf16 cast anyway).
- bf16 throughout; f32 accumulation in PSUM and for LN stats rows.
- Group-local AllToAll (4 cores per batch) ships y^T bf16; tail
  (proj+LN2+MLP) is row-sharded 512 tokens/core with LN folded into the
  matmuls via host-precomputed rank-1 corrections.
"""

import numpy as np

import concourse.bacc as bacc
import concourse.mybir as mybir
from concourse.tile import TileContext
from concourse.bass_utils import run_bass_kernel_spmd

F32 = mybir.dt.float32
BF16 = mybir.dt.bfloat16
AF = mybir.ActivationFunctionType
ALU = mybir.AluOpType

B, T, C, H, HD = 2, 2048, 384, 6, 64
CP1 = C + 1
N_CORES = 8
NT = T // 128   # 16
NC4 = T // 512  # 4
EPS = 1e-5

_COMPILED = {}


def build_program():
    nc = bacc.Bacc(trn_type="TRN2", num_devices=N_CORES)
    mm = nc.tensor.matmul

    def din(name, shape, dt=F32):
        return nc.dram_tensor(name, list(shape), dt, kind="ExternalInput")

    xT_d = din("xT", (3, 128, T), BF16)
    wqk_d = din("wqk", (128, 6 * 128), BF16)
    wv_d = din("wv", (128, 3 * 128), BF16)
    r1qk_d = din("r1qk", (1, 512), BF16)
    r1v_d = din("r1v", (1, 256), BF16)
    onesT_d = din("onesT", (1, T), BF16)
    c1qk_d = din("c1qk", (128, 2))
    c1v_d = din("c1v", (128, 1))
    ident_d = din("ident", (64, 64), BF16)
    onesc_d = din("onesc", (128, 1), BF16)
    ones65_d = din("ones65", (65, 128), BF16)
    tcol_d = din("tcol", (1, 1))
    sbias_d = din("sbias", (1, 2))
    epsc_d = din("epsc", (1, 1))
    wproj_d = din("wproj", (2, 64, 24 * 128), BF16)
    bproj_d = din("bproj", (128, 3))
    wf_d = din("wf", (128, 36 * 128), BF16)
    r2f_d = din("r2f", (1, 1536), BF16)
    c2b_d = din("c2b", (128, 12))
    wf2_d = din("wf2", (128, 36 * 128), BF16)
    bfc2_d = din("bfc2", (128, 3))
    out_d = nc.dram_tensor("oT", [C, 512], F32, kind="ExternalOutput")

    with TileContext(nc) as tc, nc.allow_low_precision(reason="bf16 kernel, validated ~6e-3 < 2e-2 gate"):
        with (
            tc.tile_pool(name="const", bufs=1) as cpool,
            tc.tile_pool(name="dram", bufs=1, space="DRAM") as dpool,
            tc.tile_pool(name="ps_mm", bufs=2, space="PSUM") as ppm,
            tc.tile_pool(name="ps_z", bufs=2, space="PSUM") as ppz,
            tc.tile_pool(name="ps_y", bufs=2, space="PSUM") as ppy,
            tc.tile_pool(name="qk", bufs=1) as qkp,
        ):
            a2a_in = [dpool.tile([8, 64, 512], BF16, name=f"a2a_in{s}") for s in range(2)]
            a2a_out = [dpool.tile([8, 64, 512], BF16, name=f"a2a_out{s}") for s in range(2)]

            ident = cpool.tile([64, 64], BF16, tag="ident", name="ident")
            onesc = cpool.tile([128, 1], BF16, tag="onesc", name="onesc")
            ones65 = cpool.tile([65, 128], BF16, tag="ones65", name="ones65")
            tcol = cpool.tile([1, 1], F32, tag="tcol", name="tcol")
            sbias = cpool.tile([1, 2], F32, tag="sbias", name="sbias")
            epsc = cpool.tile([1, 1], F32, tag="epsc", name="epsc")

            # persistent per-slot activations (base-partition-0, bf16)
            qT = [qkp.tile([64, T], BF16, tag=f"qT{s}", name=f"qT{s}") for s in range(2)]
            kT = [qkp.tile([64, T], BF16, tag=f"kT{s}", name=f"kT{s}") for s in range(2)]
            vAug = [qkp.tile([128, NT * 65], BF16, tag=f"vAug{s}", name=f"vAug{s}") for s in range(2)]

            # ---------------- phase 1: LN1 stats + QKV ----------------
            with (
                tc.tile_pool(name="xt", bufs=1) as xp,
                tc.tile_pool(name="sq", bufs=2) as sqp,
                tc.tile_pool(name="st", bufs=4) as stp,
                tc.tile_pool(name="ps_tr", bufs=2, space="PSUM") as ppt,
            ):
                xT = [xp.tile([128, T], BF16, tag=f"xt{kc}", name=f"xt{kc}") for kc in range(3)]
                for kc in range(3):
                    nc.sync.dma_start(out=xT[kc][:, :], in_=xT_d[kc, :, :])
                nc.sync.dma_start(out=ident[:, :], in_=ident_d[:, :])
                nc.sync.dma_start(out=onesc[:, :], in_=onesc_d[:, :])
                nc.sync.dma_start(out=ones65[:, :], in_=ones65_d[:, :])
                nc.sync.dma_start(out=tcol[:, :], in_=tcol_d[:, :])
                nc.sync.dma_start(out=sbias[:, :], in_=sbias_d[:, :])
                nc.sync.dma_start(out=epsc[:, :], in_=epsc_d[:, :])
                wqkall = xp.tile([128, 6 * 128], BF16, tag="wqkall", name="wqkall")
                wvall = xp.tile([128, 3 * 128], BF16, tag="wvall", name="wvall")
                wqk = [[wqkall[:, (s * 3 + kc) * 128:(s * 3 + kc + 1) * 128] for kc in range(3)] for s in range(2)]
                wv = [wvall[:, kc * 128:(kc + 1) * 128] for kc in range(3)]
                r1qk = xp.tile([1, 512], BF16, tag="r1qk", name="r1qk")
                r1v = xp.tile([1, 256], BF16, tag="r1v", name="r1v")
                onesT = xp.tile([1, T], BF16, tag="onesT", name="onesT")
                nc.sync.dma_start(out=onesT[:, :], in_=onesT_d[:, :])
                c1qk = xp.tile([128, 2], F32, tag="c1qk", name="c1qk")
                c1v = xp.tile([128, 1], F32, tag="c1v", name="c1v")
                nc.sync.dma_start(out=r1qk[:, :], in_=r1qk_d[:, :])
                nc.sync.dma_start(out=r1v[:, :], in_=r1v_d[:, :])
                nc.sync.dma_start(out=c1qk[:, :], in_=c1qk_d[:, :])
                nc.sync.dma_start(out=c1v[:, :], in_=c1v_d[:, :])
                nc.sync.dma_start(out=wqkall[:, :], in_=wqk_d[:, :])
                nc.sync.dma_start(out=wvall[:, :], in_=wv_d[:, :])

                mu16 = xp.tile([1, T], BF16, tag="mu16", name="mu16")
                mu_row = xp.tile([1, T], F32, tag="mu_row", name="mu_row")
                rstd_bc = xp.tile([128, T], F32, tag="rstd_bc", name="rstd_bc")

                # ---- per-chunk stats chains pipelined & interleaved with QKV ----
                qk_c = [xp.tile([128, T], BF16, tag=f"qk_c{s}", name=f"qk_c{s}") for s in range(2)]
                v_c = xp.tile([128, T], BF16, tag="v_c", name="v_c")
                vA = [xp.tile([64, T], BF16, tag=f"vA{s}", name=f"vA{s}") for s in range(2)]
                for s in range(2):
                    nc.gpsimd.memset(vAug[s][:, :], 1.0)

                def stats_chunk(c4):
                    sl = slice(c4 * 512, (c4 + 1) * 512)
                    ps = ppz.tile([1, 512], F32, tag="z", name="z")
                    for kc in range(3):
                        mm(ps[0:1, :], onesc[:, :], xT[kc][:, sl], start=(kc == 0), stop=(kc == 2))
                    nc.scalar.activation(mu_row[0:1, sl], ps[0:1, :], AF.Identity,
                                         bias=sbias[0:1, 0:1], scale=1.0 / CP1)
                    ps2 = ppz.tile([1, 512], F32, tag="z", name="z")
                    for kc in range(3):
                        sq = sqp.tile([128, 512], BF16, tag="sq", name="sq")
                        nc.scalar.square(sq[:, :], xT[kc][:, sl])
                        mm(ps2[0:1, :], onesc[:, :], sq[:, :], start=(kc == 0), stop=(kc == 2))
                    msq = stp.tile([1, 512], F32, tag="msq", name="msq")
                    nc.scalar.activation(msq[0:1, :], ps2[0:1, :], AF.Identity,
                                         bias=sbias[0:1, 1:2], scale=1.0 / CP1)
                    var = stp.tile([1, 512], F32, tag="var", name="var")
                    nc.vector.tensor_tensor(var[0:1, :], mu_row[0:1, sl], mu_row[0:1, sl], ALU.mult)
                    nc.vector.tensor_tensor(var[0:1, :], msq[0:1, :], var[0:1, :], ALU.subtract)
                    nc.scalar.activation(var[0:1, :], var[0:1, :], AF.Sqrt, bias=epsc[0:1, 0:1])
                    rstd = stp.tile([1, 512], F32, tag="rstd", name="rstd")
                    rscr = stp.tile([1, 512], F32, tag="rscr", name="rscr")
                    nc.vector.reciprocal_approx_accurate(rstd[0:1, :], var[0:1, :], rscr[0:1, :])
                    rstd16 = stp.tile([1, 512], BF16, tag="rstd16", name="rstd16")
                    nc.vector.tensor_copy(rstd16[0:1, :], rstd[0:1, :])
                    nc.vector.tensor_copy(mu16[0:1, sl], mu_row[0:1, sl])
                    bps = ppm.tile([128, 512], F32, tag="mm", name="mm")
                    mm(bps[:, :], ones65[0:1, :], rstd16[0:1, :], start=True, stop=True)
                    nc.vector.tensor_copy(rstd_bc[:, sl], bps[:, :])

                def qkv_mat(c4, dst, lhsT_chunks, r1a, r1b, c1col):
                    sl = slice(c4 * 512, (c4 + 1) * 512)
                    ps = ppm.tile([128, 512], F32, tag="mm", name="mm")
                    for kc in range(3):
                        mm(ps[:, :], lhsT_chunks[kc], xT[kc][:, sl],
                           start=(kc == 0), stop=False)
                    mm(ps[:, :], r1a, mu16[0:1, sl], start=False, stop=False)
                    mm(ps[:, :], r1b, onesT[0:1, sl], start=False, stop=True)
                    nc.vector.tensor_tensor(dst[:, sl], ps[:, :], rstd_bc[:, sl], ALU.mult)
                    nc.vector.tensor_scalar(dst[:, sl], dst[:, sl], c1col, None, ALU.add)

                def qkv_chunk(c4):
                    sl = slice(c4 * 512, (c4 + 1) * 512)
                    for s in range(2):
                        qkv_mat(c4, qk_c[s], wqk[s],
                                r1qk[0:1, s * 128:(s + 1) * 128],
                                r1qk[0:1, 256 + s * 128:256 + (s + 1) * 128],
                                c1qk[:, s:s + 1])
                    qkv_mat(c4, v_c, wv, r1v[0:1, 0:128], r1v[0:1, 128:256], c1v[:, 0:1])
                    for s in range(2):
                        nc.sync.dma_start(out=qT[s][:, sl], in_=qk_c[s][0:64, sl])
                        nc.sync.dma_start(out=kT[s][:, sl], in_=qk_c[s][64:128, sl])
                        nc.sync.dma_start(out=vA[s][:, sl], in_=v_c[s * 64:(s + 1) * 64, sl])
                    for s in range(2):
                        for jt in range(4 * c4, 4 * c4 + 4):
                            tr = ppt.tile([128, 64], BF16, tag="tr", name="tr")
                            nc.tensor.transpose(tr[:, :], vA[s][:, jt * 128:(jt + 1) * 128], ident[:, :])
                            nc.vector.tensor_copy(vAug[s][:, jt * 65:jt * 65 + 64], tr[:, :])

                stats_chunk(0)
                stats_chunk(1)
                qkv_chunk(0)
                stats_chunk(2)
                qkv_chunk(1)
                stats_chunk(3)
                qkv_chunk(2)
                qkv_chunk(3)

            # ---------------- phase 2: attention (transposed layout) ----------------
            with (
                tc.tile_pool(name="sp", bufs=1) as spp,
                tc.tile_pool(name="att", bufs=2) as amp,
                tc.tile_pool(name="attp", bufs=1) as am1,
                tc.tile_pool(name="ps_b", bufs=2, space="PSUM") as ppb,
            ):
                # spt tiles: per (slot, jt), columns = global i in [jt*128, T)
                spt = [[spp.tile([128, T - jt * 128], BF16, tag=f"spt{s}_{jt}", name=f"spt{s}_{jt}")
                        for jt in range(NT)] for s in range(2)]
                w65 = [am1.tile([65, 1], F32, tag=f"w65_{s}", name=f"w65_{s}") for s in range(2)]
                prewarm = am1.tile([1, 1], F32, tag="prewarm", name="prewarm")

                # w = colsum(vAug) over all j  (per slot, [65,1])
                for s in range(2):
                    wps = ppy.tile([65, 512], F32, tag="y", name="y")
                    for jt in range(NT):
                        mm(wps[:, 0:1], vAug[s][:, jt * 65:(jt + 1) * 65], onesc[:, :],
                           start=(jt == 0), stop=(jt == NT - 1))
                    nc.vector.tensor_copy(w65[s][:, :], wps[:, 0:1])

                for s in range(2):
                    for c4 in range(NC4):
                        ic0, ic1 = c4 * 512, (c4 + 1) * 512
                        njt = 4 * c4 + 4
                        # stage A: scores^T + exp + diag mask
                        for jt in range(njt):
                            j0 = jt * 128
                            lo = max(ic0, j0)
                            w_ = ic1 - lo
                            ps = ppm.tile([128, 512], F32, tag="mm", name="mm")
                            mm(ps[:, 0:w_], kT[s][:, j0:j0 + 128], qT[s][:, lo:ic1],
                               start=True, stop=True)
                            dst = spt[s][jt][:, lo - j0:ic1 - j0]
                            nc.scalar.activation(dst, ps[:, 0:w_], AF.Exp, scale=0.125)
                            if j0 + 128 > lo:  # tile straddles the diagonal: zero j > i
                                nc.gpsimd.affine_select(
                                    out=dst, in_=dst, compare_op=ALU.is_ge, fill=0.0,
                                    base=lo - j0, channel_multiplier=-1, pattern=[[1, w_]])
                        if c4 == 0:
                            # full path: z, normalize, expm1 (exact / poly), yAug on S'
                            zps = ppz.tile([1, 512], F32, tag="z", name="z")
                            for jt in range(njt):
                                j0 = jt * 128
                                lo = max(ic0, j0)
                                mm(zps[0:1, lo - ic0:512], onesc[:, :],
                                   spt[s][jt][:, lo - j0:ic1 - j0],
                                   start=(jt == 0), stop=(jt == njt - 1))
                            rz = amp.tile([1, 512], F32, tag="rz", name="rz")
                            nc.vector.reciprocal_approx_fast(out=rz[0:1, :], in_=zps[0:1, :])
                            rz16 = amp.tile([1, 512], BF16, tag="rz16", name="rz16")
                            nc.vector.tensor_copy(rz16[0:1, :], rz[0:1, :])
                            bps = ppb.tile([128, 512], F32, tag="bc", name="bc")
                            mm(bps[:, :], ones65[0:1, :], rz16[0:1, :], start=True, stop=True)
                            rzbc = amp.tile([128, 512], BF16, tag="rzbc", name="rzbc")
                            nc.scalar.copy(rzbc[:, :], bps[:, :])
                            for jt in range(njt):
                                j0 = jt * 128
                                lo = max(ic0, j0)
                                w_ = ic1 - lo
                                piece = spt[s][jt][:, lo - j0:ic1 - j0]
                                nc.vector.tensor_tensor(piece, piece, rzbc[:, lo - ic0:512], ALU.mult)
                                if jt == 0:
                                    ex = spt[s][0][:, 0:128]
                                    nc.scalar.activation(ex, ex, AF.Exp)
                                    nc.vector.tensor_scalar(ex, ex, -1.0, None, ALU.add)
                                    po = spt[s][0][:, 128:512]
                                    t1 = amp.tile([128, 512], BF16, tag="t1", name="t1")
                                    nc.vector.tensor_scalar(t1[:, 0:384], po, 0.5, 1.0, ALU.mult, ALU.add)
                                    nc.vector.tensor_tensor(po, po, t1[:, 0:384], ALU.mult)
                                else:
                                    t1 = amp.tile([128, 512], BF16, tag="t1", name="t1")
                                    nc.vector.tensor_scalar(t1[:, 0:w_], piece, 0.5, 1.0, ALU.mult, ALU.add)
                                    nc.vector.tensor_tensor(piece, piece, t1[:, 0:w_], ALU.mult)
                            yps = ppy.tile([65, 512], F32, tag="y", name="y")
                            for jt in range(njt):
                                j0 = jt * 128
                                lo = max(ic0, j0)
                                mm(yps[:, lo - ic0:512], vAug[s][:, jt * 65:(jt + 1) * 65],
                                   spt[s][jt][:, lo - j0:ic1 - j0],
                                   start=(jt == 0), stop=(jt == njt - 1))
                            a16 = amp.tile([65, 512], BF16, tag="a16", name="a16")
                            nc.vector.tensor_scalar(a16[64:65, :], yps[64:65, :],
                                                    float(T), -1.0 / (T * T), ALU.subtract, ALU.mult)
                            bps2 = ppb.tile([128, 512], F32, tag="bc", name="bc")
                            mm(bps2[:, :], ones65[64:65, :], a16[64:65, :], start=True, stop=True)
                            abc = amp.tile([64, 512], BF16, tag="abc", name="abc")
                            nc.scalar.copy(abc[:, :], bps2[0:64, :])
                            y16 = amp.tile([64, 512], BF16, tag="y16", name="y16")
                            nc.vector.tensor_scalar(y16[:, :], yps[0:64, :], w65[s][0:64, 0:1], None, ALU.add)
                            nc.vector.tensor_tensor(y16[:, :], y16[:, :], abc[:, :], ALU.mult)
                        else:
                            # identity region (i>=512): S' = p = e*rz, so rz factors out:
                            # yAug on e directly; row 64 = z; denom = T+1 exactly
                            yps = ppy.tile([65, 512], F32, tag="y", name="y")
                            for jt in range(njt):
                                j0 = jt * 128
                                lo = max(ic0, j0)
                                mm(yps[:, lo - ic0:512], vAug[s][:, jt * 65:(jt + 1) * 65],
                                   spt[s][jt][:, lo - j0:ic1 - j0],
                                   start=(jt == 0), stop=(jt == njt - 1))
                            z16 = amp.tile([65, 512], BF16, tag="z16_65", name="z16_65")
                            nc.vector.tensor_copy(z16[64:65, :], yps[64:65, :])
                            bps = ppb.tile([128, 512], F32, tag="bc", name="bc")
                            mm(bps[:, :], ones65[64:65, :], z16[64:65, :], start=True, stop=True)
                            rzbcf = amp.tile([128, 512], F32, tag="rzbcf", name="rzbcf")
                            nc.vector.reciprocal_approx_fast(out=rzbcf[:, :], in_=bps[:, :])
                            y16 = amp.tile([64, 512], BF16, tag="y16", name="y16")
                            nc.vector.tensor_tensor(y16[:, :], yps[0:64, :], rzbcf[0:64, :], ALU.mult)
                            nc.vector.tensor_scalar(y16[:, :], y16[:, :], w65[s][0:64, 0:1],
                                                    1.0 / (T + 1), ALU.add, ALU.mult)
                        for grp in range(2):
                            nc.sync.dma_start(out=a2a_in[s][grp * 4 + c4, :, :], in_=y16[:, :])
                    if s == 1:  # prewarm the Sqrt act-table set during the a2a wait
                        nc.scalar.activation(prewarm[0:1, :], epsc[0:1, :], AF.Sqrt)
                    # per-slot AllToAll: slot 0's transfer overlaps slot 1's compute
                    nc.gpsimd.collective_compute(
                        "AllToAll", ALU.bypass,
                        replica_groups=[list(range(N_CORES))],
                        ins=[a2a_in[s].opt()],
                        outs=[a2a_out[s].opt()],
                    )

            # ---------------- phase 4: proj + LN2 + MLP (512 tokens/core) ----------------
            with (
                tc.tile_pool(name="tail", bufs=1) as tp,
                tc.tile_pool(name="ps_b2", bufs=2, space="PSUM") as ppb2,
            ):
                wpall = tp.tile([128, 24 * 128], BF16, tag="wpall", name="wpall")
                wproj = [[wpall[:, (sl_ * 3 + ec) * 128:(sl_ * 3 + ec + 1) * 128] for ec in range(3)] for sl_ in range(8)]
                bproj = tp.tile([128, 3], F32, tag="bproj", name="bproj")
                wfall = tp.tile([128, 36 * 128], BF16, tag="wfall", name="wfall")
                wf = [[wfall[:, (jc * 3 + kc) * 128:(jc * 3 + kc + 1) * 128] for kc in range(3)] for jc in range(12)]
                r2f = tp.tile([1, 1536], BF16, tag="r2f", name="r2f")
                c2b = tp.tile([128, 12], F32, tag="c2b", name="c2b")
                wf2all = tp.tile([128, 36 * 128], BF16, tag="wf2all", name="wf2all")
                wf2 = [[wf2all[:, (ec * 12 + kc) * 128:(ec * 12 + kc + 1) * 128] for kc in range(12)] for ec in range(3)]
                bfc2 = tp.tile([128, 3], F32, tag="bfc2", name="bfc2")
                nc.sync.dma_start(out=wpall[0:64, :], in_=wproj_d[0, :, :])
                nc.sync.dma_start(out=wpall[64:128, :], in_=wproj_d[1, :, :])
                nc.sync.dma_start(out=bproj[:, :], in_=bproj_d[:, :])
                nc.sync.dma_start(out=wfall[:, :], in_=wf_d[:, :])
                nc.sync.dma_start(out=r2f[:, :], in_=r2f_d[:, :])
                nc.sync.dma_start(out=c2b[:, :], in_=c2b_d[:, :])
                nc.sync.dma_start(out=wf2all[:, :], in_=wf2_d[:, :])
                nc.sync.dma_start(out=bfc2[:, :], in_=bfc2_d[:, :])

                stk = [tp.tile([128, 512], BF16, tag=f"stk{sl_}", name=f"stk{sl_}") for sl_ in range(8)]
                for sl_ in range(8):
                    nc.sync.dma_start(out=stk[sl_][0:64, :], in_=a2a_out[0][sl_, :, :])
                    nc.sync.dma_start(out=stk[sl_][64:128, :], in_=a2a_out[1][sl_, :, :])

                hT = [tp.tile([128, 512], BF16, tag=f"ht{ec}", name=f"ht{ec}") for ec in range(3)]
                mu2ps = ppz.tile([1, 512], F32, tag="z", name="z")
                msq2ps = ppz.tile([1, 512], F32, tag="z", name="z")
                for ec in range(3):
                    ps = ppm.tile([128, 512], F32, tag="mm", name="mm")
                    for sl_ in range(8):
                        mm(ps[:, :], wproj[sl_][ec], stk[sl_][:, :],
                           start=(sl_ == 0), stop=(sl_ == 7))
                    nc.scalar.activation(hT[ec][:, :], ps[:, :], AF.Identity,
                                         bias=bproj[:, ec:ec + 1], scale=1.0)
                    mm(mu2ps[0:1, :], onesc[:, :], hT[ec][:, :], start=(ec == 0), stop=(ec == 2))
                    scr2 = tp.tile([128, 512], BF16, tag="scr2", name="scr2")
                    nc.scalar.square(scr2[:, :], hT[ec][:, :])
                    mm(msq2ps[0:1, :], onesc[:, :], scr2[:, :], start=(ec == 0), stop=(ec == 2))
                mu2r = tp.tile([1, 512], F32, tag="mu2r", name="mu2r")
                nc.scalar.activation(mu2r[0:1, :], mu2ps[0:1, :], AF.Identity,
                                     bias=sbias[0:1, 0:1], scale=1.0 / CP1)
                msq2r = tp.tile([1, 512], F32, tag="msq2r", name="msq2r")
                nc.scalar.activation(msq2r[0:1, :], msq2ps[0:1, :], AF.Identity,
                                     bias=sbias[0:1, 1:2], scale=1.0 / CP1)
                v2r = tp.tile([1, 512], F32, tag="v2r", name="v2r")
                nc.vector.tensor_tensor(v2r[0:1, :], mu2r[0:1, :], mu2r[0:1, :], ALU.mult)
                nc.vector.tensor_tensor(v2r[0:1, :], msq2r[0:1, :], v2r[0:1, :], ALU.subtract)
                nc.scalar.activation(v2r[0:1, :], v2r[0:1, :], AF.Sqrt, bias=epsc[0:1, 0:1])
                rstd2r = tp.tile([1, 512], F32, tag="rstd2r", name="rstd2r")
                r2scr = tp.tile([1, 512], F32, tag="r2scr", name="r2scr")
                nc.vector.reciprocal_approx_accurate(rstd2r[0:1, :], v2r[0:1, :], r2scr[0:1, :])
                rstd216 = tp.tile([1, 512], BF16, tag="rstd216", name="rstd216")
                nc.vector.tensor_copy(rstd216[0:1, :], rstd2r[0:1, :])
                mu216 = tp.tile([1, 512], BF16, tag="mu216", name="mu216")
                nc.vector.tensor_copy(mu216[0:1, :], mu2r[0:1, :])
                psm = ppb2.tile([128, 512], F32, tag="bc2", name="bc2")
                mm(psm[:, :], ones65[0:1, :], mu216[0:1, :], start=True, stop=True)
                mu2bc = tp.tile([128, 512], F32, tag="mu2bc", name="mu2bc")
                nc.scalar.copy(mu2bc[:, :], psm[:, :])
                ps = ppb2.tile([128, 512], F32, tag="bc2", name="bc2")
                mm(ps[:, :], ones65[0:1, :], rstd216[0:1, :], start=True, stop=True)
                rstd2bc = tp.tile([128, 512], F32, tag="rstd2bc", name="rstd2bc")
                nc.vector.tensor_copy(rstd2bc[:, :], ps[:, :])
                hs = [tp.tile([128, 512], BF16, tag=f"hs{ec}", name=f"hs{ec}") for ec in range(3)]
                for ec in range(3):
                    hd = tp.tile([128, 512], BF16, tag="hd", name="hd")
                    nc.vector.tensor_tensor(hd[:, :], hT[ec][:, :], mu2bc[:, :], ALU.subtract)
                    nc.vector.tensor_tensor(hs[ec][:, :], hd[:, :], rstd2bc[:, :], ALU.mult)

                mT = [tp.tile([128, 512], BF16, tag=f"mt{jc}", name=f"mt{jc}") for jc in range(12)]
                for jc in range(12):
                    ps = ppm.tile([128, 512], F32, tag="mm", name="mm")
                    for kc in range(3):
                        mm(ps[:, :], wf[jc][kc], hs[kc][:, :], start=(kc == 0), stop=False)
                    mm(ps[:, :], r2f[0:1, jc * 128:(jc + 1) * 128], rstd216[0:1, :], start=False, stop=True)
                    nc.scalar.activation(mT[jc][:, :], ps[:, :], AF.Gelu,
                                         bias=c2b[:, jc:jc + 1], scale=1.0)
                for ec in range(3):
                    ps = ppm.tile([128, 512], F32, tag="mm", name="mm")
                    for kc in range(12):
                        mm(ps[:, :], wf2[ec][kc], mT[kc][:, :],
                           start=(kc == 0), stop=(kc == 11))
                    oT = tp.tile([128, 512], F32, tag=f"ot{ec}", name=f"ot{ec}")
                    nc.scalar.activation(oT[:, :], ps[:, :], AF.Identity,
                                         bias=bfc2[:, ec:ec + 1], scale=1.0)
                    nc.sync.dma_start(out=out_d[ec * 128:(ec + 1) * 128, :], in_=oT[:, :])

    nc.compile()
    return nc


def host_prep(inputs):
    import ml_dtypes
    bf16 = ml_dtypes.bfloat16

    x = np.asarray(inputs["x"], np.float32)
    t = float(np.asarray(inputs["t"]).reshape(-1)[0])
    w1 = np.asarray(inputs["ln1_w"], np.float32); b1 = np.asarray(inputs["ln1_b"], np.float32)
    Wa = np.asarray(inputs["attn_w"], np.float32); ba = np.asarray(inputs["attn_b"], np.float32)
    Wp_ = w1[:, None] * Wa
    c1 = b1 @ Wa + ba
    Wa_main, Wa_trow = Wp_[:C], Wp_[C]
    s1 = Wp_[:C].sum(axis=0)
    w2 = np.asarray(inputs["ln2_w"], np.float32); b2 = np.asarray(inputs["ln2_b"], np.float32)
    Wf = np.asarray(inputs["fc_w"], np.float32); bf_ = np.asarray(inputs["fc_b"], np.float32)
    Wf_p = w2[:, None] * Wf
    c2 = b2 @ Wf + bf_
    Wf_main, Wf_trow = Wf_p[:C], Wf_p[C]
    s2f = Wf_p[:C].sum(axis=0)
    Wpj = np.asarray(inputs["proj_w"], np.float32); bpj = np.asarray(inputs["proj_b"], np.float32)
    Wf2 = np.asarray(inputs["fc2_w"], np.float32); bf2 = np.asarray(inputs["fc2_b"], np.float32)

    common = {
        "ident": np.eye(64, dtype=bf16),
        "onesc": np.ones((128, 1), bf16),
        "ones65": np.ones((65, 128), bf16),
        "tcol": np.full((1, 1), t, np.float32),
        "sbias": np.array([[t / CP1, t * t / CP1]], np.float32),
        "epsc": np.full((1, 1), EPS, np.float32),
        "bproj": bpj.reshape(3, 128).T.astype(np.float32).copy(),
        "c2b": c2.reshape(12, 128).T.astype(np.float32).copy(),
        "bfc2": bf2.reshape(3, 128).T.astype(np.float32).copy(),
        "r2f": np.ascontiguousarray((t * Wf_trow)[None, :]).astype(bf16),
        "onesT": np.ones((1, T), bf16),
        "wf": np.concatenate([Wf_main[kc * 128:(kc + 1) * 128, jc * 128:(jc + 1) * 128]
                              for jc in range(12) for kc in range(3)], axis=1).astype(bf16),
        "wf2": np.concatenate([Wf2[kc * 128:(kc + 1) * 128, ec * 128:(ec + 1) * 128]
                               for ec in range(3) for kc in range(12)], axis=1).astype(bf16),
    }

    # in-group rank g -> (slot0 head, slot1 head); None = dummy slot
    SLOT_HEADS = {0: (0, 1), 1: (2, 3), 2: (4, None), 3: (5, None)}
    # head -> (sender in-group rank, sender slot)
    HEAD_SRC = {0: (0, 0), 1: (0, 1), 2: (1, 0), 3: (1, 1), 4: (2, 0), 5: (3, 0)}

    in_maps = []
    for c in range(N_CORES):
        b, g = c // 4, c % 4
        m = dict(common)
        m["xT"] = np.ascontiguousarray(x[b].T).astype(bf16).reshape(3, 128, T)
        wproj = np.zeros((2, 64, 24 * 128), np.float32)
        for h in range(H):
            sr, sslot = HEAD_SRC[h]
            for ec in range(3):
                blk = Wpj[h * HD:(h + 1) * HD, ec * 128:(ec + 1) * 128]
                blkc = ((4 * b + sr) * 3 + ec) * 128
                wproj[sslot, :, blkc:blkc + 128] = blk
        m["wproj"] = wproj.astype(bf16)
        wqk = np.zeros((2, 3, 128, 128), np.float32)
        r1qk = np.zeros((1, 512), np.float32)
        c1qk = np.zeros((128, 2), np.float32)
        wv = np.zeros((3, 128, 128), np.float32)
        r1v = np.zeros((1, 256), np.float32)
        c1v = np.zeros((128, 1), np.float32)
        for s in range(2):
            h = SLOT_HEADS[g][s]
            if h is None:
                continue
            cq = slice(h * HD, (h + 1) * HD)
            ck = slice(C + h * HD, C + (h + 1) * HD)
            cv = slice(2 * C + h * HD, 2 * C + (h + 1) * HD)
            for kc in range(3):
                wqk[s, kc, :, 0:64] = Wa_main[kc * 128:(kc + 1) * 128, cq]
                wqk[s, kc, :, 64:128] = Wa_main[kc * 128:(kc + 1) * 128, ck]
                wv[kc, :, s * 64:(s + 1) * 64] = Wa_main[kc * 128:(kc + 1) * 128, cv]
            base = s * 128
            r1qk[0, base:base + 64] = -(Wa_trow + s1)[cq]
            r1qk[0, base + 64:base + 128] = -(Wa_trow + s1)[ck]
            r1qk[0, 256 + base:256 + base + 64] = t * Wa_trow[cq]
            r1qk[0, 256 + base + 64:256 + base + 128] = t * Wa_trow[ck]
            r1v[0, s * 64:(s + 1) * 64] = -(Wa_trow + s1)[cv]
            r1v[0, 128 + s * 64:128 + (s + 1) * 64] = t * Wa_trow[cv]
            c1qk[0:64, s] = c1[cq]; c1qk[64:128, s] = c1[ck]
            c1v[s * 64:(s + 1) * 64, 0] = c1[cv]
        m["wqk"] = np.concatenate([wqk[s, kc] for s in range(2) for kc in range(3)],
                                  axis=1).astype(bf16)
        m["r1qk"] = r1qk.astype(bf16); m["c1qk"] = c1qk
        m["wv"] = np.concatenate([wv[kc] for kc in range(3)], axis=1).astype(bf16)
        m["r1v"] = r1v.astype(bf16); m["c1v"] = c1v
        in_maps.append(m)
    return in_maps


def kernel(**inputs):
    if "nc" not in _COMPILED:
        _COMPILED["nc"] = build_program()
    nc = _COMPILED["nc"]
    in_maps = host_prep(inputs)
    res = run_bass_kernel_spmd(nc, in_maps, list(range(N_CORES)))
    out = np.zeros((B, T, C), np.float32)
    for c in range(N_CORES):
        oT = res.results[c]["oT"]
        b, t0 = c // 4, (c % 4) * 512
        out[b, t0:t0 + 512, :] = oT.T
    return out


# revision 24
# speedup vs baseline: 1.2285x; 1.0265x over previous
"""Trainium2 Bass kernel for nn_Block_87428354277599 (sinkhorn-attention transformer block).

Self-contained: hardcodes shapes/sharding. kernel(**inputs) -> (2, 2048, 384) f32.

Design (8 cores, SPMD, uniform program):
- 12 (batch, head) units on 16 slots: cores 0-3 = batch 0, 4-7 = batch 1;
  in-group rank g slots: g0:(h0,h1) g1:(h2,h3) g2:(h4,-) g3:(h5,-);
  '-' slots run on zero weights, output masked at the receiver's folded
  projection weights.
- Sinkhorn on S = exp(row-softmax(causal scores)) converges after ONE
  u-update (validated ~2e-4): pi = S / rowsum(S). With S' = S-1 strictly
  lower-triangular:  y_i = (w + S'@v)_i / (T + rowsum(S')_i),  w = colsum(v).
- Everything stays in the TRANSPOSED layout (partition = key j, free =
  query i): scores^T via swapped matmul, z = rowsum(e) via PE ones-matvec,
  softmax-normalize via PE-broadcast rz + vector multiply, S' = expm1(p)
  via a 2-op vector polynomial p*(1+p/2) (exact scalar exp for i<128 where
  p can be ~1), numerator AND denominator fused into one PE pass by
  augmenting v with a ones column. No PE transposes of S, no DRAM bounces.
- 1/(T+r') is linearized to (T-r')/T^2 (r' <= ~3, error 2e-6); 1/z uses
  the single-op DVE approx reciprocal (feeds a bf16 cast anyway).
- bf16 throughout; f32 accumulation in PSUM and for LN stats rows.
- Group-local AllToAll (4 cores per batch) ships y^T bf16; tail
  (proj+LN2+MLP) is row-sharded 512 tokens/core with LN folded into the
  matmuls via host-precomputed rank-1 corrections.
"""

import numpy as np

import concourse.bacc as bacc
import concourse.mybir as mybir
from concourse.tile import TileContext
from concourse.bass_utils import run_bass_kernel_spmd

F32 = mybir.dt.float32
BF16 = mybir.dt.bfloat16
AF = mybir.ActivationFunctionType
ALU = mybir.AluOpType

B, T, C, H, HD = 2, 2048, 384, 6, 64
CP1 = C + 1
N_CORES = 8
NT = T // 128   # 16
NC4 = T // 512  # 4
EPS = 1e-5

_COMPILED = {}


def build_program():
    nc = bacc.Bacc(trn_type="TRN2", num_devices=N_CORES)
    mm = nc.tensor.matmul

    def din(name, shape, dt=F32):
        return nc.dram_tensor(name, list(shape), dt, kind="ExternalInput")

    xT_d = din("xT", (3, 128, T), BF16)
    wqk_d = din("wqk", (128, 6 * 128), BF16)
    wv_d = din("wv", (128, 3 * 128), BF16)
    r1qk_d = din("r1qk", (1, 512), BF16)
    r1v_d = din("r1v", (1, 256), BF16)
    onesT_d = din("onesT", (1, T), BF16)
    c1qk_d = din("c1qk", (128, 2))
    c1v_d = din("c1v", (128, 1))
    ident_d = din("ident", (64, 64), BF16)
    onesc_d = din("onesc", (128, 1), BF16)
    ones65_d = din("ones65", (65, 128), BF16)
    tcol_d = din("tcol", (1, 1))
    sbias_d = din("sbias", (1, 2))
    epsc_d = din("epsc", (1, 1))
    wproj_d = din("wproj", (2, 64, 24 * 128), BF16)
    bproj_d = din("bproj", (128, 3))
    wf_d = din("wf", (128, 36 * 128), BF16)
    r2f_d = din("r2f", (1, 1536), BF16)
    c2b_d = din("c2b", (128, 12))
    wf2_d = din("wf2", (128, 36 * 128), BF16)
    bfc2_d = din("bfc2", (128, 3))
    out_d = nc.dram_tensor("oT", [C, 512], F32, kind="ExternalOutput")

    with TileContext(nc) as tc, nc.allow_low_precision(reason="bf16 kernel, validated ~6e-3 < 2e-2 gate"):
        with (
            tc.tile_pool(name="const", bufs=1) as cpool,
            tc.tile_pool(name="dram", bufs=1, space="DRAM") as dpool,
            tc.tile_pool(name="ps_mm", bufs=2, space="PSUM") as ppm,
            tc.tile_pool(name="ps_z", bufs=2, space="PSUM") as ppz,
            tc.tile_pool(name="ps_y", bufs=2, space="PSUM") as ppy,
            tc.tile_pool(name="qk", bufs=1) as qkp,
        ):
            a2a_in = [dpool.tile([8, 64, 512], BF16, name=f"a2a_in{s}") for s in range(2)]
            a2a_out = [dpool.tile([8, 64, 512], BF16, name=f"a2a_out{s}") for s in range(2)]

            ident = cpool.tile([64, 64], BF16, tag="ident", name="ident")
            onesc = cpool.tile([128, 1], BF16, tag="onesc", name="onesc")
            ones65 = cpool.tile([65, 128], BF16, tag="ones65", name="ones65")
            tcol = cpool.tile([1, 1], F32, tag="tcol", name="tcol")
            sbias = cpool.tile([1, 2], F32, tag="sbias", name="sbias")
            epsc = cpool.tile([1, 1], F32, tag="epsc", name="epsc")

            # persistent per-slot activations (base-partition-0, bf16)
            qT = [qkp.tile([64, T], BF16, tag=f"qT{s}", name=f"qT{s}") for s in range(2)]
            kT = [qkp.tile([64, T], BF16, tag=f"kT{s}", name=f"kT{s}") for s in range(2)]
            vAug = [qkp.tile([128, NT * 65], BF16, tag=f"vAug{s}", name=f"vAug{s}") for s in range(2)]

            # ---------------- phase 1: LN1 stats + QKV ----------------
            with (
                tc.tile_pool(name="xt", bufs=1) as xp,
                tc.tile_pool(name="sq", bufs=2) as sqp,
                tc.tile_pool(name="st", bufs=4) as stp,
                tc.tile_pool(name="ps_tr", bufs=2, space="PSUM") as ppt,
            ):
                xT = [xp.tile([128, T], BF16, tag=f"xt{kc}", name=f"xt{kc}") for kc in range(3)]
                for kc in range(3):
                    nc.sync.dma_start(out=xT[kc][:, :], in_=xT_d[kc, :, :])
                nc.sync.dma_start(out=ident[:, :], in_=ident_d[:, :])
                nc.sync.dma_start(out=onesc[:, :], in_=onesc_d[:, :])
                nc.sync.dma_start(out=ones65[:, :], in_=ones65_d[:, :])
                nc.sync.dma_start(out=tcol[:, :], in_=tcol_d[:, :])
                nc.sync.dma_start(out=sbias[:, :], in_=sbias_d[:, :])
                nc.sync.dma_start(out=epsc[:, :], in_=epsc_d[:, :])
                wqkall = xp.tile([128, 6 * 128], BF16, tag="wqkall", name="wqkall")
                wvall = xp.tile([128, 3 * 128], BF16, tag="wvall", name="wvall")
                wqk = [[wqkall[:, (s * 3 + kc) * 128:(s * 3 + kc + 1) * 128] for kc in range(3)] for s in range(2)]
                wv = [wvall[:, kc * 128:(kc + 1) * 128] for kc in range(3)]
                r1qk = xp.tile([1, 512], BF16, tag="r1qk", name="r1qk")
                r1v = xp.tile([1, 256], BF16, tag="r1v", name="r1v")
                onesT = xp.tile([1, T], BF16, tag="onesT", name="onesT")
                nc.sync.dma_start(out=onesT[:, :], in_=onesT_d[:, :])
                c1qk = xp.tile([128, 2], F32, tag="c1qk", name="c1qk")
                c1v = xp.tile([128, 1], F32, tag="c1v", name="c1v")
                nc.sync.dma_start(out=r1qk[:, :], in_=r1qk_d[:, :])
                nc.sync.dma_start(out=r1v[:, :], in_=r1v_d[:, :])
                nc.sync.dma_start(out=c1qk[:, :], in_=c1qk_d[:, :])
                nc.sync.dma_start(out=c1v[:, :], in_=c1v_d[:, :])
                nc.sync.dma_start(out=wqkall[:, :], in_=wqk_d[:, :])
                nc.sync.dma_start(out=wvall[:, :], in_=wv_d[:, :])

                mu16 = xp.tile([1, T], BF16, tag="mu16", name="mu16")
                mu_row = xp.tile([1, T], F32, tag="mu_row", name="mu_row")
                rstd_bc = xp.tile([128, T], F32, tag="rstd_bc", name="rstd_bc")

                # ---- per-chunk stats chains pipelined & interleaved with QKV ----
                qk_c = [xp.tile([128, T], BF16, tag=f"qk_c{s}", name=f"qk_c{s}") for s in range(2)]
                v_c = xp.tile([128, T], BF16, tag="v_c", name="v_c")
                vA = [xp.tile([64, T], BF16, tag=f"vA{s}", name=f"vA{s}") for s in range(2)]
                for s in range(2):
                    nc.gpsimd.memset(vAug[s][:, :], 1.0)

                def stats_chunk(c4):
                    sl = slice(c4 * 512, (c4 + 1) * 512)
                    ps = ppz.tile([1, 512], F32, tag="z", name="z")
                    for kc in range(3):
                        mm(ps[0:1, :], onesc[:, :], xT[kc][:, sl], start=(kc == 0), stop=(kc == 2))
                    nc.scalar.activation(mu_row[0:1, sl], ps[0:1, :], AF.Identity,
                                         bias=sbias[0:1, 0:1], scale=1.0 / CP1)
                    ps2 = ppz.tile([1, 512], F32, tag="z", name="z")
                    for kc in range(3):
                        sq = sqp.tile([128, 512], BF16, tag="sq", name="sq")
                        nc.scalar.square(sq[:, :], xT[kc][:, sl])
                        mm(ps2[0:1, :], onesc[:, :], sq[:, :], start=(kc == 0), stop=(kc == 2))
                    msq = stp.tile([1, 512], F32, tag="msq", name="msq")
                    nc.scalar.activation(msq[0:1, :], ps2[0:1, :], AF.Identity,
                                         bias=sbias[0:1, 1:2], scale=1.0 / CP1)
                    var = stp.tile([1, 512], F32, tag="var", name="var")
                    nc.vector.tensor_tensor(var[0:1, :], mu_row[0:1, sl], mu_row[0:1, sl], ALU.mult)
                    nc.vector.tensor_tensor(var[0:1, :], msq[0:1, :], var[0:1, :], ALU.subtract)
                    nc.scalar.activation(var[0:1, :], var[0:1, :], AF.Sqrt, bias=epsc[0:1, 0:1])
                    rstd = stp.tile([1, 512], F32, tag="rstd", name="rstd")
                    rscr = stp.tile([1, 512], F32, tag="rscr", name="rscr")
                    nc.vector.reciprocal_approx_accurate(rstd[0:1, :], var[0:1, :], rscr[0:1, :])
                    rstd16 = stp.tile([1, 512], BF16, tag="rstd16", name="rstd16")
                    nc.vector.tensor_copy(rstd16[0:1, :], rstd[0:1, :])
                    nc.vector.tensor_copy(mu16[0:1, sl], mu_row[0:1, sl])
                    bps = ppm.tile([128, 512], F32, tag="mm", name="mm")
                    mm(bps[:, :], ones65[0:1, :], rstd16[0:1, :], start=True, stop=True)
                    nc.vector.tensor_copy(rstd_bc[:, sl], bps[:, :])

                def qkv_mat(c4, dst, lhsT_chunks, r1a, r1b, c1col):
                    sl = slice(c4 * 512, (c4 + 1) * 512)
                    ps = ppm.tile([128, 512], F32, tag="mm", name="mm")
                    for kc in range(3):
                        mm(ps[:, :], lhsT_chunks[kc], xT[kc][:, sl],
                           start=(kc == 0), stop=False)
                    mm(ps[:, :], r1a, mu16[0:1, sl], start=False, stop=False)
                    mm(ps[:, :], r1b, onesT[0:1, sl], start=False, stop=True)
                    nc.vector.tensor_tensor(dst[:, sl], ps[:, :], rstd_bc[:, sl], ALU.mult)
                    nc.vector.tensor_scalar(dst[:, sl], dst[:, sl], c1col, None, ALU.add)

                def qkv_chunk(c4):
                    sl = slice(c4 * 512, (c4 + 1) * 512)
                    for s in range(2):
                        qkv_mat(c4, qk_c[s], wqk[s],
                                r1qk[0:1, s * 128:(s + 1) * 128],
                                r1qk[0:1, 256 + s * 128:256 + (s + 1) * 128],
                                c1qk[:, s:s + 1])
                    qkv_mat(c4, v_c, wv, r1v[0:1, 0:128], r1v[0:1, 128:256], c1v[:, 0:1])
                    for s in range(2):
                        nc.sync.dma_start(out=qT[s][:, sl], in_=qk_c[s][0:64, sl])
                        nc.sync.dma_start(out=kT[s][:, sl], in_=qk_c[s][64:128, sl])
                        nc.sync.dma_start(out=vA[s][:, sl], in_=v_c[s * 64:(s + 1) * 64, sl])
                    for s in range(2):
                        for jt in range(4 * c4, 4 * c4 + 4):
                            tr = ppt.tile([128, 64], BF16, tag="tr", name="tr")
                            nc.tensor.transpose(tr[:, :], vA[s][:, jt * 128:(jt + 1) * 128], ident[:, :])
                            nc.vector.tensor_copy(vAug[s][:, jt * 65:jt * 65 + 64], tr[:, :])

                stats_chunk(0)
                stats_chunk(1)
                qkv_chunk(0)
                stats_chunk(2)
                qkv_chunk(1)
                stats_chunk(3)
                qkv_chunk(2)
                qkv_chunk(3)

            # ---------------- phase 2: attention (transposed layout) ----------------
            with (
                tc.tile_pool(name="sp", bufs=1) as spp,
                tc.tile_pool(name="att", bufs=2) as amp,
                tc.tile_pool(name="attp", bufs=1) as am1,
                tc.tile_pool(name="ps_b", bufs=2, space="PSUM") as ppb,
            ):
                # spt tiles: per (slot, jt), columns = global i in [jt*128, T)
                spt = [[spp.tile([128, T - jt * 128], BF16, tag=f"spt{s}_{jt}", name=f"spt{s}_{jt}")
                        for jt in range(NT)] for s in range(2)]
                w65 = [am1.tile([65, 1], F32, tag=f"w65_{s}", name=f"w65_{s}") for s in range(2)]
                prewarm = am1.tile([1, 1], F32, tag="prewarm", name="prewarm")

                # w = colsum(vAug) over all j  (per slot, [65,1])
                for s in range(2):
                    wps = ppy.tile([65, 512], F32, tag="y", name="y")
                    for jt in range(NT):
                        mm(wps[:, 0:1], vAug[s][:, jt * 65:(jt + 1) * 65], onesc[:, :],
                           start=(jt == 0), stop=(jt == NT - 1))
                    nc.vector.tensor_copy(w65[s][:, :], wps[:, 0:1])

                for s in range(2):
                    for c4 in range(NC4):
                        ic0, ic1 = c4 * 512, (c4 + 1) * 512
                        njt = 4 * c4 + 4
                        # stage A: scores^T + exp + diag mask
                        for jt in range(njt):
                            j0 = jt * 128
                            lo = max(ic0, j0)
                            w_ = ic1 - lo
                            ps = ppm.tile([128, 512], F32, tag="mm", name="mm")
                            mm(ps[:, 0:w_], kT[s][:, j0:j0 + 128], qT[s][:, lo:ic1],
                               start=True, stop=True)
                            dst = spt[s][jt][:, lo - j0:ic1 - j0]
                            nc.scalar.activation(dst, ps[:, 0:w_], AF.Exp, scale=0.125)
                            if j0 + 128 > lo:  # tile straddles the diagonal: zero j > i
                                nc.gpsimd.affine_select(
                                    out=dst, in_=dst, compare_op=ALU.is_ge, fill=0.0,
                                    base=lo - j0, channel_multiplier=-1, pattern=[[1, w_]])
                        if c4 == 0:
                            # full path: z, normalize, expm1 (exact / poly), yAug on S'
                            zps = ppz.tile([1, 512], F32, tag="z", name="z")
                            for jt in range(njt):
                                j0 = jt * 128
                                lo = max(ic0, j0)
                                mm(zps[0:1, lo - ic0:512], onesc[:, :],
                                   spt[s][jt][:, lo - j0:ic1 - j0],
                                   start=(jt == 0), stop=(jt == njt - 1))
                            rz = amp.tile([1, 512], F32, tag="rz", name="rz")
                            nc.vector.reciprocal_approx_fast(out=rz[0:1, :], in_=zps[0:1, :])
                            rz16 = amp.tile([1, 512], BF16, tag="rz16", name="rz16")
                            nc.vector.tensor_copy(rz16[0:1, :], rz[0:1, :])
                            bps = ppb.tile([128, 512], F32, tag="bc", name="bc")
                            mm(bps[:, :], ones65[0:1, :], rz16[0:1, :], start=True, stop=True)
                            rzbc = amp.tile([128, 512], BF16, tag="rzbc", name="rzbc")
                            nc.scalar.copy(rzbc[:, :], bps[:, :])
                            for jt in range(njt):
                                j0 = jt * 128
                                lo = max(ic0, j0)
                                w_ = ic1 - lo
                                piece = spt[s][jt][:, lo - j0:ic1 - j0]
                                nc.vector.tensor_tensor(piece, piece, rzbc[:, lo - ic0:512], ALU.mult)
                                if jt == 0:
                                    ex = spt[s][0][:, 0:128]
                                    nc.scalar.activation(ex, ex, AF.Exp)
                                    nc.vector.tensor_scalar(ex, ex, -1.0, None, ALU.add)
                                    po = spt[s][0][:, 128:512]
                                    t1 = amp.tile([128, 512], BF16, tag="t1", name="t1")
                                    nc.vector.tensor_scalar(t1[:, 0:384], po, 0.5, 1.0, ALU.mult, ALU.add)
                                    nc.vector.tensor_tensor(po, po, t1[:, 0:384], ALU.mult)
                                else:
                                    t1 = amp.tile([128, 512], BF16, tag="t1", name="t1")
                                    nc.vector.tensor_scalar(t1[:, 0:w_], piece, 0.5, 1.0, ALU.mult, ALU.add)
                                    nc.vector.tensor_tensor(piece, piece, t1[:, 0:w_], ALU.mult)
                            yps = ppy.tile([65, 512], F32, tag="y", name="y")
                            for jt in range(njt):
                                j0 = jt * 128
                                lo = max(ic0, j0)
                                mm(yps[:, lo - ic0:512], vAug[s][:, jt * 65:(jt + 1) * 65],
                                   spt[s][jt][:, lo - j0:ic1 - j0],
                                   start=(jt == 0), stop=(jt == njt - 1))
                            a16 = amp.tile([65, 512], BF16, tag="a16", name="a16")
                            nc.vector.tensor_scalar(a16[64:65, :], yps[64:65, :],
                                                    float(T), -1.0 / (T * T), ALU.subtract, ALU.mult)
                            bps2 = ppb.tile([128, 512], F32, tag="bc", name="bc")
                            mm(bps2[:, :], ones65[64:65, :], a16[64:65, :], start=True, stop=True)
                            abc = amp.tile([64, 512], BF16, tag="abc", name="abc")
                            nc.scalar.copy(abc[:, :], bps2[0:64, :])
                            y16 = amp.tile([64, 512], BF16, tag="y16", name="y16")
                            nc.vector.tensor_scalar(y16[:, :], yps[0:64, :], w65[s][0:64, 0:1], None, ALU.add)
                            nc.vector.tensor_tensor(y16[:, :], y16[:, :], abc[:, :], ALU.mult)
                        else:
                            # identity region (i>=512): S' = p = e*rz, so rz factors out:
                            # yAug on e directly; row 64 = z; denom = T+1 exactly
                            yps = ppy.tile([65, 512], F32, tag="y", name="y")
                            for jt in range(njt):
                                j0 = jt * 128
                                lo = max(ic0, j0)
                                mm(yps[:, lo - ic0:512], vAug[s][:, jt * 65:(jt + 1) * 65],
                                   spt[s][jt][:, lo - j0:ic1 - j0],
                                   start=(jt == 0), stop=(jt == njt - 1))
                            z16 = amp.tile([65, 512], BF16, tag="z16_65", name="z16_65")
                            nc.vector.tensor_copy(z16[64:65, :], yps[64:65, :])
                            bps = ppb.tile([128, 512], F32, tag="bc", name="bc")
                            mm(bps[:, :], ones65[64:65, :], z16[64:65, :], start=True, stop=True)
                            rzbcf = amp.tile([128, 512], F32, tag="rzbcf", name="rzbcf")
                            nc.vector.reciprocal_approx_fast(out=rzbcf[:, :], in_=bps[:, :])
                            y16 = amp.tile([64, 512], BF16, tag="y16", name="y16")
                            nc.vector.tensor_tensor(y16[:, :], yps[0:64, :], rzbcf[0:64, :], ALU.mult)
                            nc.vector.tensor_scalar(y16[:, :], y16[:, :], w65[s][0:64, 0:1],
                                                    1.0 / (T + 1), ALU.add, ALU.mult)
                        for grp in range(2):
                            nc.sync.dma_start(out=a2a_in[s][grp * 4 + c4, :, :], in_=y16[:, :])
                    if s == 1:  # prewarm the Sqrt act-table set during the a2a wait
                        nc.scalar.activation(prewarm[0:1, :], epsc[0:1, :], AF.Sqrt)
                    # per-slot AllToAll: slot 0's transfer overlaps slot 1's compute
                    nc.gpsimd.collective_compute(
                        "AllToAll", ALU.bypass,
                        replica_groups=[list(range(N_CORES))],
                        ins=[a2a_in[s].opt()],
                        outs=[a2a_out[s].opt()],
                    )

            # ---------------- phase 4: proj + LN2 + MLP (512 tokens/core) ----------------
            with (
                tc.tile_pool(name="tail", bufs=1) as tp,
                tc.tile_pool(name="ps_b2", bufs=2, space="PSUM") as ppb2,
            ):
                wpall = tp.tile([128, 24 * 128], BF16, tag="wpall", name="wpall")
                wproj = [[wpall[:, (sl_ * 3 + ec) * 128:(sl_ * 3 + ec + 1) * 128] for ec in range(3)] for sl_ in range(8)]
                bproj = tp.tile([128, 3], F32, tag="bproj", name="bproj")
                wfall = tp.tile([128, 36 * 128], BF16, tag="wfall", name="wfall")
                wf = [[wfall[:, (jc * 3 + kc) * 128:(jc * 3 + kc + 1) * 128] for kc in range(3)] for jc in range(12)]
                r2f = tp.tile([1, 1536], BF16, tag="r2f", name="r2f")
                c2b = tp.tile([128, 12], F32, tag="c2b", name="c2b")
                wf2all = tp.tile([128, 36 * 128], BF16, tag="wf2all", name="wf2all")
                wf2 = [[wf2all[:, (ec * 12 + kc) * 128:(ec * 12 + kc + 1) * 128] for kc in range(12)] for ec in range(3)]
                bfc2 = tp.tile([128, 3], F32, tag="bfc2", name="bfc2")
                nc.sync.dma_start(out=wpall[0:64, :], in_=wproj_d[0, :, :])
                nc.sync.dma_start(out=wpall[64:128, :], in_=wproj_d[1, :, :])
                nc.sync.dma_start(out=bproj[:, :], in_=bproj_d[:, :])
                nc.sync.dma_start(out=wfall[:, :], in_=wf_d[:, :])
                nc.sync.dma_start(out=r2f[:, :], in_=r2f_d[:, :])
                nc.sync.dma_start(out=c2b[:, :], in_=c2b_d[:, :])
                nc.sync.dma_start(out=wf2all[:, :], in_=wf2_d[:, :])
                nc.sync.dma_start(out=bfc2[:, :], in_=bfc2_d[:, :])

                stk = [tp.tile([128, 512], BF16, tag=f"stk{sl_}", name=f"stk{sl_}") for sl_ in range(8)]
                for sl_ in range(8):
                    nc.sync.dma_start(out=stk[sl_][0:64, :], in_=a2a_out[0][sl_, :, :])
                    nc.sync.dma_start(out=stk[sl_][64:128, :], in_=a2a_out[1][sl_, :, :])

                hT = [tp.tile([128, 512], BF16, tag=f"ht{ec}", name=f"ht{ec}") for ec in range(3)]
                mu2ps = ppz.tile([1, 512], F32, tag="z", name="z")
                msq2ps = ppz.tile([1, 512], F32, tag="z", name="z")
                for ec in range(3):
                    ps = ppm.tile([128, 512], F32, tag="mm", name="mm")
                    for sl_ in range(8):
                        mm(ps[:, :], wproj[sl_][ec], stk[sl_][:, :],
                           start=(sl_ == 0), stop=(sl_ == 7))
                    nc.scalar.activation(hT[ec][:, :], ps[:, :], AF.Identity,
                                         bias=bproj[:, ec:ec + 1], scale=1.0)
                    mm(mu2ps[0:1, :], onesc[:, :], hT[ec][:, :], start=(ec == 0), stop=(ec == 2))
                    scr2 = tp.tile([128, 512], BF16, tag="scr2", name="scr2")
                    nc.scalar.square(scr2[:, :], hT[ec][:, :])
                    mm(msq2ps[0:1, :], onesc[:, :], scr2[:, :], start=(ec == 0), stop=(ec == 2))
                mu2r = tp.tile([1, 512], F32, tag="mu2r", name="mu2r")
                nc.scalar.activation(mu2r[0:1, :], mu2ps[0:1, :], AF.Identity,
                                     bias=sbias[0:1, 0:1], scale=1.0 / CP1)
                msq2r = tp.tile([1, 512], F32, tag="msq2r", name="msq2r")
                nc.scalar.activation(msq2r[0:1, :], msq2ps[0:1, :], AF.Identity,
                                     bias=sbias[0:1, 1:2], scale=1.0 / CP1)
                v2r = tp.tile([1, 512], F32, tag="v2r", name="v2r")
                nc.vector.tensor_tensor(v2r[0:1, :], mu2r[0:1, :], mu2r[0:1, :], ALU.mult)
                nc.vector.tensor_tensor(v2r[0:1, :], msq2r[0:1, :], v2r[0:1, :], ALU.subtract)
                nc.scalar.activation(v2r[0:1, :], v2r[0:1, :], AF.Sqrt, bias=epsc[0:1, 0:1])
                rstd2r = tp.tile([1, 512], F32, tag="rstd2r", name="rstd2r")
                r2scr = tp.tile([1, 512], F32, tag="r2scr", name="r2scr")
                nc.vector.reciprocal_approx_accurate(rstd2r[0:1, :], v2r[0:1, :], r2scr[0:1, :])
                rstd216 = tp.tile([1, 512], BF16, tag="rstd216", name="rstd216")
                nc.vector.tensor_copy(rstd216[0:1, :], rstd2r[0:1, :])
                mu216 = tp.tile([1, 512], BF16, tag="mu216", name="mu216")
                nc.vector.tensor_copy(mu216[0:1, :], mu2r[0:1, :])
                psm = ppb2.tile([128, 512], F32, tag="bc2", name="bc2")
                mm(psm[:, :], ones65[0:1, :], mu216[0:1, :], start=True, stop=True)
                mu2bc = tp.tile([128, 512], F32, tag="mu2bc", name="mu2bc")
                nc.scalar.copy(mu2bc[:, :], psm[:, :])
                ps = ppb2.tile([128, 512], F32, tag="bc2", name="bc2")
                mm(ps[:, :], ones65[0:1, :], rstd216[0:1, :], start=True, stop=True)
                rstd2bc = tp.tile([128, 512], F32, tag="rstd2bc", name="rstd2bc")
                nc.vector.tensor_copy(rstd2bc[:, :], ps[:, :])
                hs = [tp.tile([128, 512], BF16, tag=f"hs{ec}", name=f"hs{ec}") for ec in range(3)]
                for ec in range(3):
                    hd = tp.tile([128, 512], BF16, tag="hd", name="hd")
                    nc.vector.tensor_tensor(hd[:, :], hT[ec][:, :], mu2bc[:, :], ALU.subtract)
                    nc.vector.tensor_tensor(hs[ec][:, :], hd[:, :], rstd2bc[:, :], ALU.mult)

                mT = [tp.tile([128, 512], BF16, tag=f"mt{jc}", name=f"mt{jc}") for jc in range(12)]
                for jc in range(12):
                    ps = ppm.tile([128, 512], F32, tag="mm", name="mm")
                    for kc in range(3):
                        mm(ps[:, :], wf[jc][kc], hs[kc][:, :], start=(kc == 0), stop=False)
                    mm(ps[:, :], r2f[0:1, jc * 128:(jc + 1) * 128], rstd216[0:1, :], start=False, stop=True)
                    nc.scalar.activation(mT[jc][:, :], ps[:, :], AF.Gelu,
                                         bias=c2b[:, jc:jc + 1], scale=1.0)
                for ec in range(3):
                    ps = ppm.tile([128, 512], F32, tag="mm", name="mm")
                    for kc in range(12):
                        mm(ps[:, :], wf2[ec][kc], mT[kc][:, :],
                           start=(kc == 0), stop=(kc == 11))
                    oT = tp.tile([128, 512], F32, tag=f"ot{ec}", name=f"ot{ec}")
                    nc.scalar.activation(oT[:, :], ps[:, :], AF.Identity,
                                         bias=bfc2[:, ec:ec + 1], scale=1.0)
                    nc.sync.dma_start(out=out_d[ec * 128:(ec + 1) * 128, :], in_=oT[:, :])

    nc.compile()
    return nc


def host_prep(inputs):
    import ml_dtypes
    bf16 = ml_dtypes.bfloat16

    x = np.asarray(inputs["x"], np.float32)
    t = float(np.asarray(inputs["t"]).reshape(-1)[0])
    w1 = np.asarray(inputs["ln1_w"], np.float32); b1 = np.asarray(inputs["ln1_b"], np.float32)
    Wa = np.asarray(inputs["attn_w"], np.float32); ba = np.asarray(inputs["attn_b"], np.float32)
    Wp_ = w1[:, None] * Wa
    c1 = b1 @ Wa + ba
    Wa_main, Wa_trow = Wp_[:C], Wp_[C]
    s1 = Wp_[:C].sum(axis=0)
    w2 = np.asarray(inputs["ln2_w"], np.float32); b2 = np.asarray(inputs["ln2_b"], np.float32)
    Wf = np.asarray(inputs["fc_w"], np.float32); bf_ = np.asarray(inputs["fc_b"], np.float32)
    Wf_p = w2[:, None] * Wf
    c2 = b2 @ Wf + bf_
    Wf_main, Wf_trow = Wf_p[:C], Wf_p[C]
    s2f = Wf_p[:C].sum(axis=0)
    Wpj = np.asarray(inputs["proj_w"], np.float32); bpj = np.asarray(inputs["proj_b"], np.float32)
    Wf2 = np.asarray(inputs["fc2_w"], np.float32); bf2 = np.asarray(inputs["fc2_b"], np.float32)

    common = {
        "ident": np.eye(64, dtype=bf16),
        "onesc": np.ones((128, 1), bf16),
        "ones65": np.ones((65, 128), bf16),
        "tcol": np.full((1, 1), t, np.float32),
        "sbias": np.array([[t / CP1, t * t / CP1]], np.float32),
        "epsc": np.full((1, 1), EPS, np.float32),
        "bproj": bpj.reshape(3, 128).T.astype(np.float32).copy(),
        "c2b": c2.reshape(12, 128).T.astype(np.float32).copy(),
        "bfc2": bf2.reshape(3, 128).T.astype(np.float32).copy(),
        "r2f": np.ascontiguousarray((t * Wf_trow)[None, :]).astype(bf16),
        "onesT": np.ones((1, T), bf16),
        "wf": np.concatenate([Wf_main[kc * 128:(kc + 1) * 128, jc * 128:(jc + 1) * 128]
                              for jc in range(12) for kc in range(3)], axis=1).astype(bf16),
        "wf2": np.concatenate([Wf2[kc * 128:(kc + 1) * 128, ec * 128:(ec + 1) * 128]
                               for ec in range(3) for kc in range(12)], axis=1).astype(bf16),
    }

    # in-group rank g -> (slot0 head, slot1 head); None = dummy slot
    SLOT_HEADS = {0: (0, 1), 1: (2, 3), 2: (4, None), 3: (5, None)}
    # head -> (sender in-group rank, sender slot)
    HEAD_SRC = {0: (0, 0), 1: (0, 1), 2: (1, 0), 3: (1, 1), 4: (2, 0), 5: (3, 0)}

    in_maps = []
    for c in range(N_CORES):
        b, g = c // 4, c % 4
        m = dict(common)
        m["xT"] = np.ascontiguousarray(x[b].T).astype(bf16).reshape(3, 128, T)
        wproj = np.zeros((2, 64, 24 * 128), np.float32)
        for h in range(H):
            sr, sslot = HEAD_SRC[h]
            for ec in range(3):
                blk = Wpj[h * HD:(h + 1) * HD, ec * 128:(ec + 1) * 128]
                blkc = ((4 * b + sr) * 3 + ec) * 128
                wproj[sslot, :, blkc:blkc + 128] = blk
        m["wproj"] = wproj.astype(bf16)
        wqk = np.zeros((2, 3, 128, 128), np.float32)
        r1qk = np.zeros((1, 512), np.float32)
        c1qk = np.zeros((128, 2), np.float32)
        wv = np.zeros((3, 128, 128), np.float32)
        r1v = np.zeros((1, 256), np.float32)
        c1v = np.zeros((128, 1), np.float32)
        for s in range(2):
            h = SLOT_HEADS[g][s]
            if h is None:
                continue
            cq = slice(h * HD, (h + 1) * HD)
            ck = slice(C + h * HD, C + (h + 1) * HD)
            cv = slice(2 * C + h * HD, 2 * C + (h + 1) * HD)
            for kc in range(3):
                wqk[s, kc, :, 0:64] = Wa_main[kc * 128:(kc + 1) * 128, cq]
                wqk[s, kc, :, 64:128] = Wa_main[kc * 128:(kc + 1) * 128, ck]
                wv[kc, :, s * 64:(s + 1) * 64] = Wa_main[kc * 128:(kc + 1) * 128, cv]
            base = s * 128
            r1qk[0, base:base + 64] = -(Wa_trow + s1)[cq]
            r1qk[0, base + 64:base + 128] = -(Wa_trow + s1)[ck]
            r1qk[0, 256 + base:256 + base + 64] = t * Wa_trow[cq]
            r1qk[0, 256 + base + 64:256 + base + 128] = t * Wa_trow[ck]
            r1v[0, s * 64:(s + 1) * 64] = -(Wa_trow + s1)[cv]
            r1v[0, 128 + s * 64:128 + (s + 1) * 64] = t * Wa_trow[cv]
            c1qk[0:64, s] = c1[cq]; c1qk[64:128, s] = c1[ck]
            c1v[s * 64:(s + 1) * 64, 0] = c1[cv]
        m["wqk"] = np.concatenate([wqk[s, kc] for s in range(2) for kc in range(3)],
                                  axis=1).astype(bf16)
        m["r1qk"] = r1qk.astype(bf16); m["c1qk"] = c1qk
        m["wv"] = np.concatenate([wv[kc] for kc in range(3)], axis=1).astype(bf16)
        m["r1v"] = r1v.astype(bf16); m["c1v"] = c1v
        in_maps.append(m)
    return in_maps


def kernel(**inputs):
    if "nc" not in _COMPILED:
        _COMPILED["nc"] = build_program()
    nc = _COMPILED["nc"]
    in_maps = host_prep(inputs)
    res = run_bass_kernel_spmd(nc, in_maps, list(range(N_CORES)))
    out = np.zeros((B, T, C), np.float32)
    for c in range(N_CORES):
        oT = res.results[c]["oT"]
        b, t0 = c // 4, (c % 4) * 512
        out[b, t0:t0 + 512, :] = oT.T
    return out
